# revision 6
# baseline (speedup 1.0000x reference)
"""Trainium2 Bass kernel for GQA attention (B=2, S=2048, H=2048, 32 Q heads,
8 KV heads, HD=64, RoPE, causal) with output projection.

Sharding: TP=4 over heads within each batch, DP=2 over batch -> 8 cores.
Core c handles batch c//4, head-rank c%4 (8 Q heads, 2 KV heads).
Each core computes a partial o_proj output [S, H]; the host sums the 4
partials per batch (cheaper than on-device all-reduce at these sizes).

v2: all inputs pre-cast to bf16 and pre-packed on host into [128, *]
contiguous layouts so every DMA is a plain 2D transfer straight into the
persistent SBUF tile (no staging, no on-device casts).  ACT runs exp only
(the RoPE psum->sbuf copy moved to gpsimd and the softmax-denominator
reciprocal moved to DVE, so the exp table set never reloads).  AV matmuls
and exp are causally trimmed on diagonal tiles (diagonal-first ki order
keeps PSUM has_written coverage correct) which removes all ep memsets.
o_proj accumulates into one [128, 2048] bf16 tile per s-tile and stores
with a single DMA; the output y is bf16 (host sums partials in fp32).

Per-core device inputs (host-packed, all bf16 except c2/ss/msk fp32):
  xall  [128, 4*16*512]  x chunk-major: chunk sc | H-tile i | s
  wqall [128, 16*512]    Wq^T H-tile-major (head order h0,h4,h1,h5,...)
  wkv   [128, 16*256]    Wk^T | Wv^T interleaved per H-tile
  wot   [128, 4*2048]    Wo^T d-tile-major
  c2/ss [128, S] RoPE cos/sin tables (two 64-row head blocks stacked)
  rot   [128, 128] rotate-half permutation, msk [128,128] causal bias
Output y [128, 16*2048] bf16: s-tile-major y rows.
"""

import numpy as np
from contextlib import ExitStack

import concourse.bass as bass
import concourse.bacc as bacc
import concourse.mybir as mybir
import concourse.tile as tile
from concourse.bass_utils import run_bass_kernel_spmd

F32 = mybir.dt.float32
BF16 = mybir.dt.bfloat16
AF = mybir.ActivationFunctionType

B, S, H = 2, 2048, 2048
NH, NKV, HD = 32, 8, 64
TP = 4                      # head-parallel ranks per batch
NQO = NH // TP * HD         # 512 per-core q features (8 heads)
NKO = NKV // TP * HD        # 128 per-core kv features (2 heads)
NHL = NH // TP              # 8 local q heads
EXP_SCALE = 1.0 / 8.0       # 1/sqrt(HD)
MASK_VAL = -30000.0
P = 128
QC = 512                    # q-chunk (one PSUM bank of fp32)
NSC = S // QC               # 4 q/s chunks
NPT = S // P                # 16 partition tiles of S
NHT = H // P                # 16 partition tiles of H


def build_nc():
    nc = bacc.Bacc("TRN2", target_bir_lowering=False, debug=False, num_devices=8)

    xall = nc.dram_tensor("xall", [P, NSC * NHT * QC], BF16, kind="ExternalInput").ap()
    wqall = nc.dram_tensor("wqall", [P, NHT * NQO], BF16, kind="ExternalInput").ap()
    wkv = nc.dram_tensor("wkv", [P, NHT * 2 * NKO], BF16, kind="ExternalInput").ap()
    wot = nc.dram_tensor("wot", [P, 4 * S], BF16, kind="ExternalInput").ap()
    c2 = nc.dram_tensor("c2", [P, S], F32, kind="ExternalInput").ap()
    ss = nc.dram_tensor("ss", [P, S], F32, kind="ExternalInput").ap()
    msk = nc.dram_tensor("msk", [P, P], F32, kind="ExternalInput").ap()
    rot = nc.dram_tensor("rot", [P, P], BF16, kind="ExternalInput").ap()
    y = nc.dram_tensor("y", [P, NPT * S], BF16, kind="ExternalOutput").ap()

    with tile.TileContext(nc) as tc, ExitStack() as ctx:
        persist = ctx.enter_context(tc.tile_pool(name="persist", bufs=1))
        xpool = ctx.enter_context(tc.tile_pool(name="xpool", bufs=3))
        p1 = ctx.enter_context(tc.tile_pool(name="p1", bufs=2))
        p2 = ctx.enter_context(tc.tile_pool(name="p2", bufs=6))
        p2a = ctx.enter_context(tc.tile_pool(name="p2a", bufs=2))
        p3 = ctx.enter_context(tc.tile_pool(name="p3", bufs=2))
        # all 8 PSUM banks shared across projection + attention + o_proj:
        #   tag "sp"  [128, 1024] x2 (4 banks): score pairs, o_proj, Q/K proj
        #   tag "aux" [128, 512]  x4 (4 banks): AV accum, recip bcast, rope rot, V
        psum = ctx.enter_context(tc.tile_pool(name="psum", bufs=2, space="PSUM"))

        # ---- persistent tiles ----
        c2_sb = persist.tile([P, S], F32, tag="c2", name="c2sb")
        ss_sb = persist.tile([P, S], F32, tag="ss", name="sssb")
        msk_sb = persist.tile([P, P], F32, tag="msk", name="msksb")
        rot_sb = persist.tile([P, P], BF16, tag="rot", name="rotsb")
        ones65 = persist.tile([65, 64], F32, tag="ones65", name="ones65")
        ones65b = persist.tile([65, 64], BF16, tag="ones65b", name="ones65b")

        wq_sb = persist.tile([P, NHT * NQO], BF16, tag="wq", name="wqsb")
        wkv_sb = persist.tile([P, NHT * 2 * NKO], BF16, tag="wkv", name="wkvsb")
        wot_sb = persist.tile([P, 4 * S], BF16, tag="wot", name="wotsb")

        qtbc = [[persist.tile([P, QC], BF16, tag=f"qtbc{t}_{sc}", name=f"qtbc{t}_{sc}")
                 for sc in range(NSC)] for t in range(4)]
        ktbc = [persist.tile([P, QC], BF16, tag=f"ktbc{sc}", name=f"ktbc{sc}")
                for sc in range(NSC)]
        vaug = [persist.tile([P, 130], BF16, tag=f"vaug{i}", name=f"vaug{i}")
                for i in range(NPT)]
        atbc = [[persist.tile([P, QC], BF16, tag=f"atbc{t}_{qc}", name=f"atbc{t}_{qc}")
                 for qc in range(NSC)] for t in range(4)]

        def wqt(i, t):         # Wq^T tile i, head-pair column block t
            return wq_sb[:, NQO * i + P * t: NQO * i + P * (t + 1)]

        def wqt_full(i):
            return wq_sb[:, NQO * i: NQO * (i + 1)]

        def wkt(i):
            return wkv_sb[:, 2 * NKO * i: 2 * NKO * i + NKO]

        def wvt(i):
            return wkv_sb[:, 2 * NKO * i + NKO: 2 * NKO * (i + 1)]

        def wott(t, oc):       # Wo^T d-tile t, output H-chunk oc
            return wot_sb[:, S * t + QC * oc: S * t + QC * (oc + 1)]

        # ---- input DMAs: first-needed-first, round-robin the two queues ----
        xchunk = [None] * NSC

        def load_xchunk(sc, eng):
            xc = xpool.tile([P, NHT * QC], BF16, tag="xchunk", name=f"xchunk{sc}")
            half = NHT * QC // 2
            eng.dma_start(xc[:, 0:half], xall[:, NHT * QC * sc: NHT * QC * sc + half])
            oth = nc.gpsimd if eng is nc.sync else nc.sync
            oth.dma_start(xc[:, half:], xall[:, NHT * QC * sc + half: NHT * QC * (sc + 1)])
            xchunk[sc] = xc

        load_xchunk(0, nc.sync)
        nc.gpsimd.dma_start(wq_sb[:, 0: NHT * NQO // 2], wqall[:, 0: NHT * NQO // 2])
        nc.sync.dma_start(wq_sb[:, NHT * NQO // 2:], wqall[:, NHT * NQO // 2:])
        nc.gpsimd.dma_start(rot_sb[:], rot[:])
        nc.sync.dma_start(c2_sb[:], c2[:])
        nc.gpsimd.dma_start(ss_sb[:], ss[:])
        nc.sync.dma_start(wkv_sb[:], wkv[:])
        nc.gpsimd.dma_start(msk_sb[:], msk[:])
        nc.gpsimd.memset(ones65[64:65, :], 1.0)
        nc.gpsimd.memset(ones65b[64:65, :], 1.0)
        load_xchunk(1, nc.gpsimd)
        nc.sync.dma_start(wot_sb[:, 0: 2 * S], wot[:, 0: 2 * S])
        nc.gpsimd.dma_start(wot_sb[:, 2 * S:], wot[:, 2 * S:])

        def xt(i, sc):
            return xchunk[sc][:, QC * i: QC * (i + 1)]

        def rope_tile(dst_ap, ps, sc):
            """RoPE: dst = raw*C2 + (R @ raw)*SS for one [128, 512] chunk."""
            ssl = slice(QC * sc, QC * (sc + 1))
            raw = p1.tile([P, QC], BF16, tag="rope_raw")
            nc.scalar.copy(raw[:], ps[:])
            rps = psum.tile([P, QC], F32, tag="aux", bufs=4, name="rps")
            nc.tensor.matmul(rps[:], lhsT=rot_sb[:], rhs=raw[:],
                             start=True, stop=True)
            t1 = p1.tile([P, QC], F32, tag="rope_t1")
            nc.gpsimd.tensor_mul(t1[:], raw[:], c2_sb[:, ssl])
            t2 = p1.tile([P, QC], F32, tag="rope_t2")
            nc.vector.tensor_mul(t2[:], rps[:], ss_sb[:, ssl])
            nc.vector.tensor_add(dst_ap, t1[:], t2[:])

        def attn_step(hp, qc, ki, avpA, avpB, first, last):
            """One [128-k x 512-q] step for a head pair.  Diagonal tiles are
            causally trimmed: scores/exp/AV only touch cols >= col0."""
            j = ki - 4 * qc
            col0 = P * j if j >= 0 else 0
            kc = P * (ki % 4)
            sp = psum.tile([P, 2 * QC], F32, tag="sp", bufs=2, name="sp")
            nc.tensor.matmul(
                sp[:, col0:QC],
                lhsT=ktbc[ki // 4][0:64, kc:kc + P],
                rhs=qtbc[hp][qc][0:64, col0:QC],
                start=True, stop=True,
            )
            nc.tensor.matmul(
                sp[:, QC + col0:2 * QC],
                lhsT=ktbc[ki // 4][64:128, kc:kc + P],
                rhs=qtbc[hp][qc][64:128, col0:QC],
                start=True, stop=True,
            )
            ep = p2.tile([P, 2 * QC], BF16, tag="ep")
            if j >= 0:
                nc.vector.tensor_add(sp[:, col0:col0 + P],
                                     sp[:, col0:col0 + P], msk_sb[:])
                nc.vector.tensor_add(sp[:, QC + col0:QC + col0 + P],
                                     sp[:, QC + col0:QC + col0 + P], msk_sb[:])
                nc.scalar.activation(ep[:, col0:QC], sp[:, col0:QC],
                                     AF.Exp, scale=EXP_SCALE)
                nc.scalar.activation(ep[:, QC + col0:2 * QC],
                                     sp[:, QC + col0:2 * QC],
                                     AF.Exp, scale=EXP_SCALE)
            else:
                nc.scalar.activation(ep[:], sp[:], AF.Exp, scale=EXP_SCALE)
            nc.tensor.matmul(
                avpA[:, col0:QC], lhsT=vaug[ki][:, 0:65], rhs=ep[:, col0:QC],
                start=first, stop=last,
            )
            nc.tensor.matmul(
                avpB[:, col0:QC], lhsT=vaug[ki][:, 65:130],
                rhs=ep[:, QC + col0:2 * QC],
                start=first, stop=last,
            )

        def normalize(hp, off, qc, avp):
            rcpf = p2a.tile([65, QC], F32, tag="rcpf")
            nc.vector.reciprocal(rcpf[64:65, :], avp[64:65, :])
            rcp = p2a.tile([65, QC], BF16, tag="rcp")
            nc.vector.tensor_copy(rcp[64:65, :], rcpf[64:65, :])
            atrs = p2a.tile([64, QC], F32, tag="atrs")
            nc.vector.tensor_copy(atrs[:], avp[0:64, :])
            rbc = psum.tile([64, QC], F32, tag="aux", bufs=4, name="rbc")
            nc.tensor.matmul(rbc[:], lhsT=ones65b[64:65, 0:64],
                             rhs=rcp[64:65, :], start=True, stop=True)
            nc.vector.tensor_mul(atbc[hp][qc][off:off + 64, :],
                                 atrs[:], rbc[:])

        def oproj_piece(qc, st):
            stj = st - 4 * qc
            ost = p3.tile([P, S], BF16, tag="ost")
            for oc in range(NSC):
                op = psum.tile([P, QC], F32, tag="sp", bufs=2, name="op")
                for ft in range(4):
                    nc.tensor.matmul(
                        op[:],
                        lhsT=atbc[ft][qc][:, P * stj:P * (stj + 1)],
                        rhs=wott(ft, oc),
                        start=(ft == 0), stop=(ft == 3),
                    )
                if oc % 2 == 0:
                    nc.scalar.copy(ost[:, QC * oc:QC * (oc + 1)], op[:])
                else:
                    nc.vector.tensor_copy(ost[:, QC * oc:QC * (oc + 1)], op[:])
            nc.sync.dma_start(y[:, S * st:S * (st + 1)], ost[:])

        from collections import deque
        oproj_q = deque()
        pending_norms = []
        for sc in range(NSC):
            if sc + 2 < NSC:
                load_xchunk(sc + 2, nc.sync if sc % 2 == 0 else nc.gpsimd)
            # Q^T chunks
            for t in range(4):
                ps = psum.tile([P, QC], F32, tag="sp", bufs=2, name="qkps")
                for i in range(NHT):
                    nc.tensor.matmul(
                        ps[:], lhsT=wqt(i, t), rhs=xt(i, sc),
                        start=(i == 0), stop=(i == NHT - 1),
                    )
                rope_tile(qtbc[t][sc][:], ps, sc)
            # K^T chunk
            ps = psum.tile([P, QC], F32, tag="sp", bufs=2, name="qkps")
            for i in range(NHT):
                nc.tensor.matmul(
                    ps[:], lhsT=wkt(i), rhs=xt(i, sc),
                    start=(i == 0), stop=(i == NHT - 1),
                )
            rope_tile(ktbc[sc][:], ps, sc)
            # V tiles in this chunk
            for j in range(4 * sc, 4 * sc + 4):
                jj = j - 4 * sc
                ps = psum.tile([P, NKO], F32, tag="aux", bufs=4, name="vps")
                for i in range(NHT):
                    nc.tensor.matmul(
                        ps[:], lhsT=xt(i, sc)[:, P * jj:P * (jj + 1)],
                        rhs=wvt(i),
                        start=(i == 0), stop=(i == NHT - 1),
                    )
                nc.vector.tensor_copy(vaug[j][:, 0:64], ps[:, 0:64])
                nc.vector.tensor_copy(vaug[j][:, 65:129], ps[:, 64:128])
                nc.gpsimd.memset(vaug[j][:, 64:65], 1.0)
                nc.gpsimd.memset(vaug[j][:, 129:130], 1.0)

            # ---- attention for qc = sc (causal: only needs chunks <= sc) --
            # ki order: diagonal j=0 first (full-width, start=True covers the
            # whole avp bank), then j=1..3 trimmed, then off-diagonals.
            qc = sc
            ki_order = [4 * qc + j for j in range(4)] + list(range(4 * qc))
            for hp in range(4):
                avpA = psum.tile([65, QC], F32, tag="aux", bufs=4, name="avpA")
                avpB = psum.tile([65, QC], F32, tag="aux", bufs=4, name="avpB")
                for n, ki in enumerate(ki_order):
                    attn_step(hp, qc, ki, avpA, avpB,
                              n == 0, n == len(ki_order) - 1)
                # deferred work from the previous group fills PE while this
                # group's exps/AVs drain
                prev, pending_norms = pending_norms, [
                    (hp, 0, qc, avpA), (hp, 64, qc, avpB)]
                for args in prev:
                    normalize(*args)
                if oproj_q:
                    oproj_piece(*oproj_q.popleft())
            for st in range(4 * qc, 4 * qc + 4):
                oproj_q.append((qc, st))
        for args in pending_norms:
            normalize(*args)
        while oproj_q:
            oproj_piece(*oproj_q.popleft())

    nc.compile()
    return nc


def _host_tables():
    inv_freq = 1.0 / (10000.0 ** (np.arange(0, HD, 2, dtype=np.float32) / HD))
    pos = np.arange(S, dtype=np.float32)
    freqs = np.einsum('s,d->sd', pos, inv_freq)          # [S, 32]
    emb = np.concatenate([freqs, freqs], axis=-1)        # [S, 64]
    cosT = np.cos(emb).T.astype(np.float32)              # [64, S]
    sinT = np.sin(emb).T.astype(np.float32)
    c2 = np.ascontiguousarray(np.vstack([cosT, cosT]))   # [128, S]
    ss = np.ascontiguousarray(np.vstack([sinT, sinT]))
    # rotate-half as a matmul: out[d] = sum_d' R[d', d] * in[d']
    R64 = np.zeros((HD, HD), dtype=np.float32)
    for d in range(32):
        R64[d + 32, d] = -1.0       # out[d] = -in[d+32]
        R64[d, d + 32] = 1.0        # out[d+32] = in[d]
    rot = np.zeros((P, P), dtype=np.float32)
    rot[0:64, 0:64] = R64
    rot[64:128, 64:128] = R64
    # causal bias for a diagonal 128x128 tile in scores^T[k, q] layout
    kk = np.arange(P)[:, None]
    qq = np.arange(P)[None, :]
    msk = np.where(kk <= qq, 0.0, MASK_VAL).astype(np.float32)
    import ml_dtypes
    rot = rot.astype(ml_dtypes.bfloat16)   # exact: entries are 0/+-1
    return c2, ss, rot, msk


# q/o head order within a rank block: pair heads (u, u+4) in each 128-row tile
_HEAD_ORDER = [0, 4, 1, 5, 2, 6, 3, 7]


def _pack128(a):
    """[128*n, m] row-major -> [128, n*m] with block i at cols [m*i, m*(i+1))."""
    n = a.shape[0] // P
    return np.ascontiguousarray(
        a.reshape(n, P, a.shape[1]).transpose(1, 0, 2).reshape(P, -1))


def _make_in_maps(hidden_states, Wq, Wk, Wv, Wo):
    import ml_dtypes
    BF = ml_dtypes.bfloat16
    hs = np.asarray(hidden_states, dtype=np.float32)
    Wq = np.asarray(Wq, dtype=np.float32)
    Wk = np.asarray(Wk, dtype=np.float32)
    Wv = np.asarray(Wv, dtype=np.float32)
    Wo = np.asarray(Wo, dtype=np.float32)
    c2, ss, rot, msk = _host_tables()
    in_maps = []
    for c in range(8):
        b, r = c // 4, c % 4
        # row indices of Wq (= cols of Wo) for this rank, in device head order
        qrows = np.concatenate([
            np.arange(HD) + (NHL * r + u) * HD for u in _HEAD_ORDER
        ])
        xt_ = hs[b].T.astype(BF)                          # [H, S]
        # chunk-major packing: [128, sc, i, s']
        xp = xt_.reshape(NHT, P, NSC, QC).transpose(1, 2, 0, 3).reshape(P, -1)
        wq_ = _pack128(Wq[qrows, :].T.astype(BF))         # [128, 16*512]
        wk_ = Wk[NKO * r:NKO * (r + 1), :].T.astype(BF)   # [H, 128]
        wv_ = Wv[NKO * r:NKO * (r + 1), :].T.astype(BF)
        wkv_ = _pack128(np.concatenate([wk_, wv_], axis=1))
        wot_ = _pack128(Wo[:, qrows].T.astype(BF))        # [128, 4*2048]
        in_maps.append({
            "xall": np.ascontiguousarray(xp),
            "wqall": wq_,
            "wkv": wkv_,
            "wot": wot_,
            "c2": c2, "ss": ss, "msk": msk, "rot": rot,
        })
    return in_maps


_NC = None


def _get_nc():
    global _NC
    if _NC is None:
        _NC = build_nc()
    return _NC


def run_cores(hidden_states, Wq, Wk, Wv, Wo, **run_kwargs):
    """Run the SPMD kernel; returns (out [B,S,H] fp32, BassKernelResults)."""
    nc = _get_nc()
    in_maps = _make_in_maps(hidden_states, Wq, Wk, Wv, Wo)
    res = run_bass_kernel_spmd(nc, in_maps, list(range(8)), **run_kwargs)
    out = np.zeros((B, S, H), dtype=np.float32)
    for c in range(8):
        yb = np.asarray(res.results[c]["y"], dtype=np.float32)
        out[c // 4] += yb.reshape(P, NPT, S).transpose(1, 0, 2).reshape(S, H)
    return out, res


def kernel(hidden_states, Wq, Wk, Wv, Wo):
    out, _ = run_cores(hidden_states, Wq, Wk, Wv, Wo)
    return out


# revision 11
# speedup vs baseline: 1.1317x; 1.1317x over previous
"""Trainium2 Bass kernel for GQA attention (B=2, S=2048, H=2048, 32 Q heads,
8 KV heads, HD=64, RoPE, causal) with output projection.

Sharding: TP=4 over heads within each batch, DP=2 over batch -> 8 cores.
Core c handles batch c//4, head-rank c%4 (8 Q heads, 2 KV heads).
Each core computes a partial o_proj output [S, H]; the host sums the 4
partials per batch (cheaper than on-device all-reduce at these sizes).

v2: all inputs pre-cast to bf16 and pre-packed on host into [128, *]
contiguous layouts so every DMA is a plain 2D transfer straight into the
persistent SBUF tile (no staging, no on-device casts).  ACT runs exp and
copies only (one table set, zero reloads); the softmax reciprocal uses the
custom-DVE fast Newton-Raphson op.  AV matmuls and exp are causally
trimmed on diagonal tiles (diagonal-first ki order keeps PSUM has_written
coverage correct) -- no ep memsets.  o_proj accumulates into one
[128, 2048] bf16 tile per s-tile, stored with a single DMA; y is bf16
(host sums rank partials in fp32).

v3 (this file): software-pipelined emission.  The projection work for
chunk sc+1 is a generator of small quanta that are interleaved between
the attention steps of chunk sc, so the PE stays fed while ACT crunches
the exps (the attention inner loop is ACT-latency-bound).  Normalize is
split: the DVE part (reciprocal + copies) is emitted at the head-pair
boundary, the PE broadcast matmul + final multiply are deferred past the
next head-pair's first step so they never head-of-line-block the PE.

PSUM budget (8 banks):  scores 2x[128,1024] (4) | avpA/avpB [65,512] (2)
                        | proj/oproj/rope/V/rbc rotating [128,512] (2)
"""

import numpy as np
from contextlib import ExitStack

import concourse.bass as bass
import concourse.bacc as bacc
import concourse.mybir as mybir
import concourse.tile as tile
from concourse.bass_utils import run_bass_kernel_spmd

F32 = mybir.dt.float32
BF16 = mybir.dt.bfloat16
AF = mybir.ActivationFunctionType

B, S, H = 2, 2048, 2048
NH, NKV, HD = 32, 8, 64
TP = 4                      # head-parallel ranks per batch
NQO = NH // TP * HD         # 512 per-core q features (8 heads)
NKO = NKV // TP * HD        # 128 per-core kv features (2 heads)
NHL = NH // TP              # 8 local q heads
EXP_SCALE = 1.0 / 8.0       # 1/sqrt(HD)
MASK_VAL = -30000.0
P = 128
QC = 512                    # q-chunk (one PSUM bank of fp32)
NSC = S // QC               # 4 q/s chunks
NPT = S // P                # 16 partition tiles of S
NHT = H // P                # 16 partition tiles of H


def build_nc():
    nc = bacc.Bacc("TRN2", target_bir_lowering=False, debug=False, num_devices=8)

    xall = nc.dram_tensor("xall", [P, NSC * NHT * QC], BF16, kind="ExternalInput").ap()
    wqall = nc.dram_tensor("wqall", [P, NHT * NQO], BF16, kind="ExternalInput").ap()
    wkv = nc.dram_tensor("wkv", [P, NHT * 2 * NKO], BF16, kind="ExternalInput").ap()
    wot = nc.dram_tensor("wot", [P, 4 * S], BF16, kind="ExternalInput").ap()
    c2 = nc.dram_tensor("c2", [P, S], F32, kind="ExternalInput").ap()
    ss = nc.dram_tensor("ss", [P, S], F32, kind="ExternalInput").ap()
    msk = nc.dram_tensor("msk", [P, P], F32, kind="ExternalInput").ap()
    rot = nc.dram_tensor("rot", [P, P], BF16, kind="ExternalInput").ap()
    y = nc.dram_tensor("y", [P, NPT * S], BF16, kind="ExternalOutput").ap()

    with tile.TileContext(nc) as tc, ExitStack() as ctx:
        persist = ctx.enter_context(tc.tile_pool(name="persist", bufs=1))
        xpool = ctx.enter_context(tc.tile_pool(name="xpool", bufs=3))
        p1 = ctx.enter_context(tc.tile_pool(name="p1", bufs=2))
        p2 = ctx.enter_context(tc.tile_pool(name="p2", bufs=6))
        p2a = ctx.enter_context(tc.tile_pool(name="p2a", bufs=2))
        p3 = ctx.enter_context(tc.tile_pool(name="p3", bufs=2))
        psum = ctx.enter_context(tc.tile_pool(name="psum", bufs=2, space="PSUM"))

        # ---- persistent tiles ----
        c2_sb = persist.tile([P, S], F32, tag="c2", name="c2sb")
        ss_sb = persist.tile([P, S], F32, tag="ss", name="sssb")
        msk_sb = persist.tile([P, P], F32, tag="msk", name="msksb")
        rot_sb = persist.tile([P, P], BF16, tag="rot", name="rotsb")
        ones65b = persist.tile([65, 64], BF16, tag="ones65b", name="ones65b")

        wq_sb = persist.tile([P, NHT * NQO], BF16, tag="wq", name="wqsb")
        wkv_sb = persist.tile([P, NHT * 2 * NKO], BF16, tag="wkv", name="wkvsb")
        wot_sb = persist.tile([P, 4 * S], BF16, tag="wot", name="wotsb")

        qtbc = [[persist.tile([P, QC], BF16, tag=f"qtbc{t}_{sc}", name=f"qtbc{t}_{sc}")
                 for sc in range(NSC)] for t in range(4)]
        ktbc = [persist.tile([P, QC], BF16, tag=f"ktbc{sc}", name=f"ktbc{sc}")
                for sc in range(NSC)]
        vaug = [persist.tile([P, 130], BF16, tag=f"vaug{i}", name=f"vaug{i}")
                for i in range(NPT)]
        atbc = [[persist.tile([P, QC], BF16, tag=f"atbc{t}_{qc}", name=f"atbc{t}_{qc}")
                 for qc in range(NSC)] for t in range(4)]

        def wqt(i, t):         # Wq^T tile i, head-pair column block t
            return wq_sb[:, NQO * i + P * t: NQO * i + P * (t + 1)]

        def wkt(i):
            return wkv_sb[:, 2 * NKO * i: 2 * NKO * i + NKO]

        def wvt(i):
            return wkv_sb[:, 2 * NKO * i + NKO: 2 * NKO * (i + 1)]

        def wott(t, oc):       # Wo^T d-tile t, output H-chunk oc
            return wot_sb[:, S * t + QC * oc: S * t + QC * (oc + 1)]

        # ---- input DMAs: first-needed-first, round-robin the two queues ----
        xchunk = [None] * NSC
        qeng = [nc.sync, nc.gpsimd]

        def load_xchunk(sc, quarters=1):
            xc = xpool.tile([P, NHT * QC], BF16, tag="xchunk", name=f"xchunk{sc}")
            w = NHT * QC // quarters
            for qq in range(quarters):
                qeng[qq % 2].dma_start(
                    xc[:, w * qq: w * (qq + 1)],
                    xall[:, NHT * QC * sc + w * qq: NHT * QC * sc + w * (qq + 1)])
            xchunk[sc] = xc

        # interleave x-chunk-0 and wq quarters so the first Q chain can
        # start as soon as the first quarter lands
        xc0 = xpool.tile([P, NHT * QC], BF16, tag="xchunk", name="xchunk0")
        xchunk[0] = xc0
        wq4 = NHT * NQO // 4
        for qq in range(4):
            nc.sync.dma_start(xc0[:, wq4 * qq: wq4 * (qq + 1)],
                              xall[:, wq4 * qq: wq4 * (qq + 1)])
            nc.gpsimd.dma_start(wq_sb[:, wq4 * qq: wq4 * (qq + 1)],
                                wqall[:, wq4 * qq: wq4 * (qq + 1)])
        nc.sync.dma_start(rot_sb[:], rot[:])
        nc.gpsimd.dma_start(wkv_sb[:], wkv[:])
        nc.sync.dma_start(c2_sb[:], c2[:])
        nc.gpsimd.dma_start(ss_sb[:], ss[:])
        nc.sync.dma_start(msk_sb[:], msk[:])
        nc.gpsimd.memset(ones65b[64:65, :], 1.0)
        load_xchunk(1, quarters=2)
        nc.sync.dma_start(wot_sb[:, 0: 2 * S], wot[:, 0: 2 * S])
        nc.gpsimd.dma_start(wot_sb[:, 2 * S:], wot[:, 2 * S:])

        def xt(i, sc):
            return xchunk[sc][:, QC * i: QC * (i + 1)]

        def rope_tile(dst_ap, ps, sc):
            """RoPE: dst = raw*C2 + (R @ raw)*SS for one [128, 512] chunk."""
            ssl = slice(QC * sc, QC * (sc + 1))
            raw = p1.tile([P, QC], BF16, tag="rope_raw")
            nc.scalar.copy(raw[:], ps[:])
            rps = psum.tile([P, QC], F32, tag="pj", bufs=2, name="rps")
            nc.tensor.matmul(rps[:], lhsT=rot_sb[:], rhs=raw[:],
                             start=True, stop=True)
            t1 = p1.tile([P, QC], F32, tag="rope_t1")
            nc.gpsimd.tensor_mul(t1[:], raw[:], c2_sb[:, ssl])
            t2 = p1.tile([P, QC], F32, tag="rope_t2")
            nc.vector.tensor_mul(t2[:], rps[:], ss_sb[:, ssl])
            nc.vector.tensor_add(dst_ap, t1[:], t2[:])

        def proj_quanta(sc):
            """Generator: all projection work for chunk sc in small quanta."""
            # Q^T chains + rope
            for t in range(4):
                ps = psum.tile([P, QC], F32, tag="pj", bufs=2, name="qkps")
                for i in range(NHT):
                    nc.tensor.matmul(
                        ps[:], lhsT=wqt(i, t), rhs=xt(i, sc),
                        start=(i == 0), stop=(i == NHT - 1),
                    )
                    if i == 7:
                        yield
                yield
                rope_tile(qtbc[t][sc][:], ps, sc)
                yield
            # K^T chain + rope
            ps = psum.tile([P, QC], F32, tag="pj", bufs=2, name="qkps")
            for i in range(NHT):
                nc.tensor.matmul(
                    ps[:], lhsT=wkt(i), rhs=xt(i, sc),
                    start=(i == 0), stop=(i == NHT - 1),
                )
                if i == 7:
                    yield
            yield
            rope_tile(ktbc[sc][:], ps, sc)
            yield
            # V tiles
            for j in range(4 * sc, 4 * sc + 4):
                jj = j - 4 * sc
                ps = psum.tile([P, NKO], F32, tag="pj", bufs=2, name="vps")
                for i in range(NHT):
                    nc.tensor.matmul(
                        ps[:], lhsT=xt(i, sc)[:, P * jj:P * (jj + 1)],
                        rhs=wvt(i),
                        start=(i == 0), stop=(i == NHT - 1),
                    )
                    if i == 7:
                        yield
                nc.vector.tensor_copy(vaug[j][:, 0:64], ps[:, 0:64])
                nc.vector.tensor_copy(vaug[j][:, 65:129], ps[:, 64:128])
                nc.gpsimd.memset(vaug[j][:, 64:65], 1.0)
                nc.gpsimd.memset(vaug[j][:, 129:130], 1.0)
                yield

        N_QUANTA = 23   # quanta per proj_quanta generator (4*3 + 3 + 4*2)

        def attn_step(hp, qc, ki, avpA, avpB, first, last):
            """One [128-k x 512-q] step for a head pair.  Diagonal tiles are
            causally trimmed: scores/exp/AV only touch cols >= col0."""
            j = ki - 4 * qc
            col0 = P * j if j >= 0 else 0
            kc = P * (ki % 4)
            sp = psum.tile([P, 2 * QC], F32, tag="sc", bufs=2, name="sp")
            nc.tensor.matmul(
                sp[:, col0:QC],
                lhsT=ktbc[ki // 4][0:64, kc:kc + P],
                rhs=qtbc[hp][qc][0:64, col0:QC],
                start=True, stop=True,
            )
            nc.tensor.matmul(
                sp[:, QC + col0:2 * QC],
                lhsT=ktbc[ki // 4][64:128, kc:kc + P],
                rhs=qtbc[hp][qc][64:128, col0:QC],
                start=True, stop=True,
            )
            ep = p2.tile([P, 2 * QC], BF16, tag="ep")
            if j >= 0:
                nc.vector.tensor_add(sp[:, col0:col0 + P],
                                     sp[:, col0:col0 + P], msk_sb[:])
                nc.vector.tensor_add(sp[:, QC + col0:QC + col0 + P],
                                     sp[:, QC + col0:QC + col0 + P], msk_sb[:])
                nc.scalar.activation(ep[:, col0:QC], sp[:, col0:QC],
                                     AF.Exp, scale=EXP_SCALE)
                nc.scalar.activation(ep[:, QC + col0:2 * QC],
                                     sp[:, QC + col0:2 * QC],
                                     AF.Exp, scale=EXP_SCALE)
            else:
                nc.scalar.activation(ep[:], sp[:], AF.Exp, scale=EXP_SCALE)
            nc.tensor.matmul(
                avpA[:, col0:QC], lhsT=vaug[ki][:, 0:65], rhs=ep[:, col0:QC],
                start=first, stop=last,
            )
            nc.tensor.matmul(
                avpB[:, col0:QC], lhsT=vaug[ki][:, 65:130],
                rhs=ep[:, QC + col0:2 * QC],
                start=first, stop=last,
            )

        def norm_pre(avp):
            """DVE part: copy AV+rowsum out of PSUM, 1/rowsum via the classic
            magic-constant seed (0x7EF311C3 - bits(x), ~5% err) plus one
            Newton-Raphson step (~0.26% err, well under the bf16 quantum)."""
            av65 = p2a.tile([65, QC], F32, tag="av65")
            nc.vector.tensor_copy(av65[:], avp[:])
            y0 = p2a.tile([65, QC], F32, tag="y0")
            nc.vector.tensor_scalar(
                y0[64:65, :].bitcast(mybir.dt.int32),
                av65[64:65, :].bitcast(mybir.dt.int32),
                -1, None, op0=mybir.AluOpType.bitwise_xor)
            nc.vector.tensor_scalar(
                y0[64:65, :].bitcast(mybir.dt.int32),
                y0[64:65, :].bitcast(mybir.dt.int32),
                0x7EF311C4, None, op0=mybir.AluOpType.add)
            nrt = p2a.tile([65, QC], F32, tag="nrt")
            nc.vector.scalar_tensor_tensor(
                nrt[64:65, :], av65[64:65, :], -1.0, y0[64:65, :],
                op0=mybir.AluOpType.mult, op1=mybir.AluOpType.mult)
            rcp = p2a.tile([65, QC], BF16, tag="rcp")
            nc.vector.scalar_tensor_tensor(
                rcp[64:65, :], nrt[64:65, :], 2.0, y0[64:65, :],
                op0=mybir.AluOpType.add, op1=mybir.AluOpType.mult)
            return rcp, av65

        def norm_post(hp, off, qc, rcp, av65):
            """PE broadcast of 1/rowsum + final normalize multiply."""
            rbc = psum.tile([64, QC], F32, tag="pj", bufs=2, name="rbc")
            nc.tensor.matmul(rbc[:], lhsT=ones65b[64:65, 0:64],
                             rhs=rcp[64:65, :], start=True, stop=True)
            nc.vector.tensor_mul(atbc[hp][qc][off:off + 64, :],
                                 av65[0:64, :], rbc[:])

        def oproj_piece(qc, st):
            stj = st - 4 * qc
            ost = p3.tile([P, S], BF16, tag="ost")
            for oc in range(NSC):
                op = psum.tile([P, QC], F32, tag="pj", bufs=2, name="op")
                for ft in range(4):
                    nc.tensor.matmul(
                        op[:],
                        lhsT=atbc[ft][qc][:, P * stj:P * (stj + 1)],
                        rhs=wott(ft, oc),
                        start=(ft == 0), stop=(ft == 3),
                    )
                if oc % 2 == 0:
                    nc.scalar.copy(ost[:, QC * oc:QC * (oc + 1)], op[:])
                else:
                    nc.vector.tensor_copy(ost[:, QC * oc:QC * (oc + 1)], op[:])
            nc.sync.dma_start(y[:, S * st:S * (st + 1)], ost[:])

        from collections import deque
        oproj_q = deque()

        # chunk 0 projections run alone (nothing to overlap with)
        for _ in proj_quanta(0):
            pass

        for sc in range(NSC):
            qc = sc
            if sc + 2 < NSC:
                load_xchunk(sc + 2, quarters=2)
            gen = proj_quanta(sc + 1) if sc + 1 < NSC else iter(())
            total_steps = 4 * (4 * qc + 4)
            pulled = 0
            step_no = 0
            pending_post = []
            ki_order = [4 * qc + j for j in range(4)] + list(range(4 * qc))
            for hp in range(4):
                avpA = psum.tile([65, QC], F32, tag="av", bufs=2, name="avpA")
                avpB = psum.tile([65, QC], F32, tag="av", bufs=2, name="avpB")
                for n, ki in enumerate(ki_order):
                    attn_step(hp, qc, ki, avpA, avpB,
                              n == 0, n == len(ki_order) - 1)
                    step_no += 1
                    if n == 0 and pending_post:
                        for args in pending_post:
                            norm_post(*args)
                        pending_post = []
                        if oproj_q:
                            oproj_piece(*oproj_q.popleft())
                    target = min(N_QUANTA, N_QUANTA * (step_no + 1) // total_steps)
                    while pulled < target:
                        if next(gen, StopIteration) is StopIteration:
                            pulled = N_QUANTA
                            break
                        pulled += 1
                rcpA, atrsA = norm_pre(avpA)
                rcpB, atrsB = norm_pre(avpB)
                pending_post = [(hp, 0, qc, rcpA, atrsA),
                                (hp, 64, qc, rcpB, atrsB)]
            for args in pending_post:
                norm_post(*args)
            for _ in gen:
                pass
            if oproj_q:
                oproj_piece(*oproj_q.popleft())
            for st in range(4 * qc, 4 * qc + 4):
                oproj_q.append((qc, st))
        while oproj_q:
            oproj_piece(*oproj_q.popleft())

    nc.compile()
    return nc


def _host_tables():
    inv_freq = 1.0 / (10000.0 ** (np.arange(0, HD, 2, dtype=np.float32) / HD))
    pos = np.arange(S, dtype=np.float32)
    freqs = np.einsum('s,d->sd', pos, inv_freq)          # [S, 32]
    emb = np.concatenate([freqs, freqs], axis=-1)        # [S, 64]
    cosT = np.cos(emb).T.astype(np.float32)              # [64, S]
    sinT = np.sin(emb).T.astype(np.float32)
    c2 = np.ascontiguousarray(np.vstack([cosT, cosT]))   # [128, S]
    ss = np.ascontiguousarray(np.vstack([sinT, sinT]))
    # rotate-half as a matmul: out[d] = sum_d' R[d', d] * in[d']
    R64 = np.zeros((HD, HD), dtype=np.float32)
    for d in range(32):
        R64[d + 32, d] = -1.0       # out[d] = -in[d+32]
        R64[d, d + 32] = 1.0        # out[d+32] = in[d]
    rot = np.zeros((P, P), dtype=np.float32)
    rot[0:64, 0:64] = R64
    rot[64:128, 64:128] = R64
    # causal bias for a diagonal 128x128 tile in scores^T[k, q] layout
    kk = np.arange(P)[:, None]
    qq = np.arange(P)[None, :]
    msk = np.where(kk <= qq, 0.0, MASK_VAL).astype(np.float32)
    import ml_dtypes
    rot = rot.astype(ml_dtypes.bfloat16)   # exact: entries are 0/+-1
    return c2, ss, rot, msk


# q/o head order within a rank block: pair heads (u, u+4) in each 128-row tile
_HEAD_ORDER = [0, 4, 1, 5, 2, 6, 3, 7]


def _pack128(a):
    """[128*n, m] row-major -> [128, n*m] with block i at cols [m*i, m*(i+1))."""
    n = a.shape[0] // P
    return np.ascontiguousarray(
        a.reshape(n, P, a.shape[1]).transpose(1, 0, 2).reshape(P, -1))


def _make_in_maps(hidden_states, Wq, Wk, Wv, Wo):
    import ml_dtypes
    BF = ml_dtypes.bfloat16
    hs = np.asarray(hidden_states, dtype=np.float32)
    Wq = np.asarray(Wq, dtype=np.float32)
    Wk = np.asarray(Wk, dtype=np.float32)
    Wv = np.asarray(Wv, dtype=np.float32)
    Wo = np.asarray(Wo, dtype=np.float32)
    c2, ss, rot, msk = _host_tables()
    in_maps = []
    for c in range(8):
        b, r = c // 4, c % 4
        # row indices of Wq (= cols of Wo) for this rank, in device head order
        qrows = np.concatenate([
            np.arange(HD) + (NHL * r + u) * HD for u in _HEAD_ORDER
        ])
        xt_ = hs[b].T.astype(BF)                          # [H, S]
        # chunk-major packing: [128, sc, i, s']
        xp = xt_.reshape(NHT, P, NSC, QC).transpose(1, 2, 0, 3).reshape(P, -1)
        wq_ = _pack128(Wq[qrows, :].T.astype(BF))         # [128, 16*512]
        wk_ = Wk[NKO * r:NKO * (r + 1), :].T.astype(BF)   # [H, 128]
        wv_ = Wv[NKO * r:NKO * (r + 1), :].T.astype(BF)
        wkv_ = _pack128(np.concatenate([wk_, wv_], axis=1))
        wot_ = _pack128(Wo[:, qrows].T.astype(BF))        # [128, 4*2048]
        in_maps.append({
            "xall": np.ascontiguousarray(xp),
            "wqall": wq_,
            "wkv": wkv_,
            "wot": wot_,
            "c2": c2, "ss": ss, "msk": msk, "rot": rot,
        })
    return in_maps


_NC = None


def _get_nc():
    global _NC
    if _NC is None:
        _NC = build_nc()
    return _NC


def run_cores(hidden_states, Wq, Wk, Wv, Wo, **run_kwargs):
    """Run the SPMD kernel; returns (out [B,S,H] fp32, BassKernelResults)."""
    nc = _get_nc()
    in_maps = _make_in_maps(hidden_states, Wq, Wk, Wv, Wo)
    res = run_bass_kernel_spmd(nc, in_maps, list(range(8)), **run_kwargs)
    out = np.zeros((B, S, H), dtype=np.float32)
    for c in range(8):
        yb = np.asarray(res.results[c]["y"], dtype=np.float32)
        out[c // 4] += yb.reshape(P, NPT, S).transpose(1, 0, 2).reshape(S, H)
    return out, res


def kernel(hidden_states, Wq, Wk, Wv, Wo):
    out, _ = run_cores(hidden_states, Wq, Wk, Wv, Wo)
    return out


# revision 15
# speedup vs baseline: 1.1863x; 1.0482x over previous
"""Trainium2 Bass kernel for GQA attention (B=2, S=2048, H=2048, 32 Q heads,
8 KV heads, HD=64, RoPE, causal) with output projection.

Sharding: TP=4 over heads within each batch, DP=2 over batch -> 8 cores.
Core c handles batch c//4, head-rank c%4 (8 Q heads, 2 KV heads).
Each core computes a partial o_proj output [S, H]; the host sums the 4
partials per batch (cheaper than on-device all-reduce at these sizes).

v2: all inputs pre-cast to bf16 and pre-packed on host into [128, *]
contiguous layouts so every DMA is a plain 2D transfer straight into the
persistent SBUF tile (no staging, no on-device casts).  ACT runs exp and
copies only (one table set, zero reloads); the softmax reciprocal uses the
custom-DVE fast Newton-Raphson op.  AV matmuls and exp are causally
trimmed on diagonal tiles (diagonal-first ki order keeps PSUM has_written
coverage correct) -- no ep memsets.  o_proj accumulates into one
[128, 2048] bf16 tile per s-tile, stored with a single DMA; y is bf16
(host sums rank partials in fp32).

v3 (this file): software-pipelined emission.  The projection work for
chunk sc+1 is a generator of small quanta that are interleaved between
the attention steps of chunk sc, so the PE stays fed while ACT crunches
the exps (the attention inner loop is ACT-latency-bound).  Normalize is
split: the DVE part (reciprocal + copies) is emitted at the head-pair
boundary, the PE broadcast matmul + final multiply are deferred past the
next head-pair's first step so they never head-of-line-block the PE.

PSUM budget (8 banks):  scores 2x[128,1024] (4) | avpA/avpB [65,512] (2)
                        | proj/oproj/rope/V/rbc rotating [128,512] (2)
"""

import numpy as np
from contextlib import ExitStack

import concourse.bass as bass
import concourse.bacc as bacc
import concourse.mybir as mybir
import concourse.tile as tile
from concourse.bass_utils import run_bass_kernel_spmd

F32 = mybir.dt.float32
BF16 = mybir.dt.bfloat16
AF = mybir.ActivationFunctionType

B, S, H = 2, 2048, 2048
NH, NKV, HD = 32, 8, 64
TP = 4                      # head-parallel ranks per batch
NQO = NH // TP * HD         # 512 per-core q features (8 heads)
NKO = NKV // TP * HD        # 128 per-core kv features (2 heads)
NHL = NH // TP              # 8 local q heads
EXP_SCALE = 1.0 / 8.0       # 1/sqrt(HD)
MASK_VAL = -30000.0
P = 128
QC = 512                    # q-chunk (one PSUM bank of fp32)
NSC = S // QC               # 4 q/s chunks
NPT = S // P                # 16 partition tiles of S
NHT = H // P                # 16 partition tiles of H


def build_nc():
    nc = bacc.Bacc("TRN2", target_bir_lowering=False, debug=False, num_devices=8)

    xall = nc.dram_tensor("xall", [P, NSC * NHT * QC], BF16, kind="ExternalInput").ap()
    wqall = nc.dram_tensor("wqall", [P, NHT * NQO], BF16, kind="ExternalInput").ap()
    wkv = nc.dram_tensor("wkv", [P, NHT * 2 * NKO], BF16, kind="ExternalInput").ap()
    wot = nc.dram_tensor("wot", [P, 4 * S], BF16, kind="ExternalInput").ap()
    c2 = nc.dram_tensor("c2", [P, S], F32, kind="ExternalInput").ap()
    ss = nc.dram_tensor("ss", [P, S], F32, kind="ExternalInput").ap()
    msk = nc.dram_tensor("msk", [P, P], F32, kind="ExternalInput").ap()
    rot = nc.dram_tensor("rot", [P, P], BF16, kind="ExternalInput").ap()
    y = nc.dram_tensor("y", [P, NPT * S], BF16, kind="ExternalOutput").ap()

    with tile.TileContext(nc) as tc, ExitStack() as ctx:
        persist = ctx.enter_context(tc.tile_pool(name="persist", bufs=1))
        xpool = ctx.enter_context(tc.tile_pool(name="xpool", bufs=3))
        p1 = ctx.enter_context(tc.tile_pool(name="p1", bufs=2))
        p2 = ctx.enter_context(tc.tile_pool(name="p2", bufs=6))
        p2a = ctx.enter_context(tc.tile_pool(name="p2a", bufs=2))
        p3 = ctx.enter_context(tc.tile_pool(name="p3", bufs=2))
        psum = ctx.enter_context(tc.tile_pool(name="psum", bufs=2, space="PSUM"))

        # ---- persistent tiles ----
        c2_sb = persist.tile([P, S], F32, tag="c2", name="c2sb")
        ss_sb = persist.tile([P, S], F32, tag="ss", name="sssb")
        msk_sb = persist.tile([P, P], F32, tag="msk", name="msksb")
        rot_sb = persist.tile([P, P], BF16, tag="rot", name="rotsb")
        ones65b = persist.tile([65, 64], BF16, tag="ones65b", name="ones65b")

        wq_sb = persist.tile([P, NHT * NQO], BF16, tag="wq", name="wqsb")
        wkv_sb = persist.tile([P, NHT * 2 * NKO], BF16, tag="wkv", name="wkvsb")
        wot_sb = persist.tile([P, 4 * S], BF16, tag="wot", name="wotsb")

        qtbc = [[persist.tile([P, QC], BF16, tag=f"qtbc{t}_{sc}", name=f"qtbc{t}_{sc}")
                 for sc in range(NSC)] for t in range(4)]
        ktbc = [persist.tile([P, QC], BF16, tag=f"ktbc{sc}", name=f"ktbc{sc}")
                for sc in range(NSC)]
        vaug = [persist.tile([P, 130], BF16, tag=f"vaug{i}", name=f"vaug{i}")
                for i in range(NPT)]
        atbc = [[persist.tile([P, QC], BF16, tag=f"atbc{t}_{qc}", name=f"atbc{t}_{qc}")
                 for qc in range(NSC)] for t in range(4)]

        def wqt(i, t):         # Wq^T tile i, head-pair column block t
            return wq_sb[:, NQO * i + P * t: NQO * i + P * (t + 1)]

        def wkt(i):
            return wkv_sb[:, 2 * NKO * i: 2 * NKO * i + NKO]

        def wvt(i):
            return wkv_sb[:, 2 * NKO * i + NKO: 2 * NKO * (i + 1)]

        def wott(t, oc):       # Wo^T d-tile t, output H-chunk oc
            return wot_sb[:, S * t + QC * oc: S * t + QC * (oc + 1)]

        # ---- input DMAs: first-needed-first, round-robin the two queues ----
        xchunk = [None] * NSC
        qeng = [nc.sync, nc.gpsimd]

        def load_xchunk(sc, quarters=1):
            xc = xpool.tile([P, NHT * QC], BF16, tag="xchunk", name=f"xchunk{sc}")
            w = NHT * QC // quarters
            for qq in range(quarters):
                qeng[qq % 2].dma_start(
                    xc[:, w * qq: w * (qq + 1)],
                    xall[:, NHT * QC * sc + w * qq: NHT * QC * sc + w * (qq + 1)])
            xchunk[sc] = xc

        # interleave x-chunk-0 and wq quarters so the first Q chain can
        # start as soon as the first quarter lands
        xc0 = xpool.tile([P, NHT * QC], BF16, tag="xchunk", name="xchunk0")
        xchunk[0] = xc0
        wq4 = NHT * NQO // 4
        for qq in range(4):
            nc.sync.dma_start(xc0[:, wq4 * qq: wq4 * (qq + 1)],
                              xall[:, wq4 * qq: wq4 * (qq + 1)])
            nc.gpsimd.dma_start(wq_sb[:, wq4 * qq: wq4 * (qq + 1)],
                                wqall[:, wq4 * qq: wq4 * (qq + 1)])
        nc.sync.dma_start(rot_sb[:], rot[:])
        nc.gpsimd.dma_start(wkv_sb[:], wkv[:])
        nc.sync.dma_start(c2_sb[:], c2[:])
        nc.gpsimd.dma_start(ss_sb[:], ss[:])
        nc.sync.dma_start(msk_sb[:], msk[:])
        nc.gpsimd.memset(ones65b[64:65, :], 1.0)
        load_xchunk(1, quarters=2)
        nc.sync.dma_start(wot_sb[:, 0: 2 * S], wot[:, 0: 2 * S])
        nc.gpsimd.dma_start(wot_sb[:, 2 * S:], wot[:, 2 * S:])

        def xt(i, sc):
            return xchunk[sc][:, QC * i: QC * (i + 1)]

        def rope_tile(dst_ap, ps, sc):
            """RoPE: dst = raw*C2 + (R @ raw)*SS for one [128, 512] chunk."""
            ssl = slice(QC * sc, QC * (sc + 1))
            raw = p1.tile([P, QC], BF16, tag="rope_raw")
            nc.scalar.copy(raw[:], ps[:])
            rps = psum.tile([P, QC], F32, tag="pj", bufs=2, name="rps")
            nc.tensor.matmul(rps[:], lhsT=rot_sb[:], rhs=raw[:],
                             start=True, stop=True)
            t1 = p1.tile([P, QC], F32, tag="rope_t1")
            nc.gpsimd.tensor_mul(t1[:], raw[:], c2_sb[:, ssl])
            t2 = p1.tile([P, QC], F32, tag="rope_t2")
            nc.vector.tensor_mul(t2[:], rps[:], ss_sb[:, ssl])
            nc.vector.tensor_add(dst_ap, t1[:], t2[:])

        def proj_quanta(sc):
            """Generator: all projection work for chunk sc in small quanta."""
            # Q^T chains + rope
            for t in range(4):
                ps = psum.tile([P, QC], F32, tag="pj", bufs=2, name="qkps")
                for i in range(NHT):
                    nc.tensor.matmul(
                        ps[:], lhsT=wqt(i, t), rhs=xt(i, sc),
                        start=(i == 0), stop=(i == NHT - 1),
                    )
                    if i == 7:
                        yield
                yield
                rope_tile(qtbc[t][sc][:], ps, sc)
                yield
            # K^T chain + rope
            ps = psum.tile([P, QC], F32, tag="pj", bufs=2, name="qkps")
            for i in range(NHT):
                nc.tensor.matmul(
                    ps[:], lhsT=wkt(i), rhs=xt(i, sc),
                    start=(i == 0), stop=(i == NHT - 1),
                )
                if i == 7:
                    yield
            yield
            rope_tile(ktbc[sc][:], ps, sc)
            yield
            # V tiles
            for j in range(4 * sc, 4 * sc + 4):
                jj = j - 4 * sc
                ps = psum.tile([P, NKO], F32, tag="pj", bufs=2, name="vps")
                for i in range(NHT):
                    nc.tensor.matmul(
                        ps[:], lhsT=xt(i, sc)[:, P * jj:P * (jj + 1)],
                        rhs=wvt(i),
                        start=(i == 0), stop=(i == NHT - 1),
                    )
                    if i == 7:
                        yield
                nc.vector.tensor_copy(vaug[j][:, 0:64], ps[:, 0:64])
                nc.vector.tensor_copy(vaug[j][:, 65:129], ps[:, 64:128])
                nc.gpsimd.memset(vaug[j][:, 64:65], 1.0)
                nc.gpsimd.memset(vaug[j][:, 129:130], 1.0)
                yield

        N_QUANTA = 23   # quanta per proj_quanta generator (4*3 + 3 + 4*2)

        def attn_step(hp, qc, ki, avpA, avpB, first, last):
            """One [128-k x 512-q] step for a head pair.  Diagonal tiles are
            causally trimmed: scores/exp/AV only touch cols >= col0."""
            j = ki - 4 * qc
            col0 = P * j if j >= 0 else 0
            kc = P * (ki % 4)
            sp = psum.tile([P, 2 * QC], F32, tag="sc", bufs=2, name="sp")
            nc.tensor.matmul(
                sp[:, col0:QC],
                lhsT=ktbc[ki // 4][0:64, kc:kc + P],
                rhs=qtbc[hp][qc][0:64, col0:QC],
                start=True, stop=True,
            )
            nc.tensor.matmul(
                sp[:, QC + col0:2 * QC],
                lhsT=ktbc[ki // 4][64:128, kc:kc + P],
                rhs=qtbc[hp][qc][64:128, col0:QC],
                start=True, stop=True,
            )
            ep = p2.tile([P, 2 * QC], BF16, tag="ep")
            if j >= 0:
                nc.vector.tensor_add(sp[:, col0:col0 + P],
                                     sp[:, col0:col0 + P], msk_sb[:])
                nc.vector.tensor_add(sp[:, QC + col0:QC + col0 + P],
                                     sp[:, QC + col0:QC + col0 + P], msk_sb[:])
                # one strided ACT op over both heads' trimmed column ranges
                spv = sp[:].rearrange("p (h w) -> p h w", h=2)
                epv = ep[:].rearrange("p (h w) -> p h w", h=2)
                nc.scalar.activation(epv[:, :, col0:QC], spv[:, :, col0:QC],
                                     AF.Exp, scale=EXP_SCALE)
            else:
                nc.scalar.activation(ep[:], sp[:], AF.Exp, scale=EXP_SCALE)
            nc.tensor.matmul(
                avpA[:, col0:QC], lhsT=vaug[ki][:, 0:65], rhs=ep[:, col0:QC],
                start=first, stop=last,
            )
            nc.tensor.matmul(
                avpB[:, col0:QC], lhsT=vaug[ki][:, 65:130],
                rhs=ep[:, QC + col0:2 * QC],
                start=first, stop=last,
            )

        def norm_pre(avp):
            """DVE part: copy AV+rowsum out of PSUM, 1/rowsum via the classic
            magic-constant seed (0x7EF311C3 - bits(x), ~5% err) plus one
            Newton-Raphson step (~0.26% err, well under the bf16 quantum)."""
            av65 = p2a.tile([65, QC], F32, tag="av65")
            nc.vector.tensor_copy(av65[:], avp[:])
            y0 = p2a.tile([65, QC], F32, tag="y0")
            nc.vector.tensor_scalar(
                y0[64:65, :].bitcast(mybir.dt.int32),
                av65[64:65, :].bitcast(mybir.dt.int32),
                -1, 0x7EF311C3, op0=mybir.AluOpType.mult,
                op1=mybir.AluOpType.add)
            nrt = p2a.tile([65, QC], F32, tag="nrt")
            nc.vector.scalar_tensor_tensor(
                nrt[64:65, :], av65[64:65, :], -1.0, y0[64:65, :],
                op0=mybir.AluOpType.mult, op1=mybir.AluOpType.mult)
            rcp = p2a.tile([65, QC], BF16, tag="rcp")
            nc.vector.scalar_tensor_tensor(
                rcp[64:65, :], nrt[64:65, :], 2.0, y0[64:65, :],
                op0=mybir.AluOpType.add, op1=mybir.AluOpType.mult)
            return rcp, av65

        def norm_post(hp, off, qc, rcp, av65):
            """PE broadcast of 1/rowsum + final normalize multiply."""
            rbc = psum.tile([64, QC], F32, tag="pj", bufs=2, name="rbc")
            nc.tensor.matmul(rbc[:], lhsT=ones65b[64:65, 0:64],
                             rhs=rcp[64:65, :], start=True, stop=True)
            nc.vector.tensor_mul(atbc[hp][qc][off:off + 64, :],
                                 av65[0:64, :], rbc[:])

        from collections import deque
        filler = deque()     # thunks of deferred PE-filler work (o_proj ocs)

        def enqueue_oproj(qc, st):
            stj = st - 4 * qc
            cell = {}

            def mk(oc):
                def th():
                    if 'ost' not in cell:
                        cell['ost'] = p3.tile([P, S], BF16, tag="ost",
                                              name=f"ost{st}")
                    ost = cell['ost']
                    op = psum.tile([P, QC], F32, tag="pj", bufs=2, name="op")
                    for ft in range(4):
                        nc.tensor.matmul(
                            op[:],
                            lhsT=atbc[ft][qc][:, P * stj:P * (stj + 1)],
                            rhs=wott(ft, oc),
                            start=(ft == 0), stop=(ft == 3),
                        )
                    if oc % 2 == 0:
                        nc.scalar.copy(ost[:, QC * oc:QC * (oc + 1)], op[:])
                    else:
                        nc.vector.tensor_copy(ost[:, QC * oc:QC * (oc + 1)], op[:])
                    if oc == NSC - 1:
                        nc.sync.dma_start(y[:, S * st:S * (st + 1)], ost[:])
                return th

            for oc in range(NSC):
                filler.append(mk(oc))

        # chunk 0 projections run alone (nothing to overlap with)
        for _ in proj_quanta(0):
            pass

        for sc in range(NSC):
            qc = sc
            if sc + 2 < NSC:
                load_xchunk(sc + 2, quarters=2)
            gen = proj_quanta(sc + 1) if sc + 1 < NSC else None
            rate = 2 if qc < 2 else 1
            pending_post = []
            ki_order = [4 * qc + j for j in range(4)] + list(range(4 * qc))
            for hp in range(4):
                avpA = psum.tile([65, QC], F32, tag="av", bufs=2, name="avpA")
                avpB = psum.tile([65, QC], F32, tag="av", bufs=2, name="avpB")
                for n, ki in enumerate(ki_order):
                    attn_step(hp, qc, ki, avpA, avpB,
                              n == 0, n == len(ki_order) - 1)
                    if n == 0 and pending_post:
                        for args in pending_post:
                            norm_post(*args)
                        pending_post = []
                    for _ in range(rate):
                        if gen is not None:
                            if next(gen, StopIteration) is StopIteration:
                                gen = None
                            else:
                                continue
                        if filler:
                            filler.popleft()()
                rcpA, atrsA = norm_pre(avpA)
                rcpB, atrsB = norm_pre(avpB)
                pending_post = [(hp, 0, qc, rcpA, atrsA),
                                (hp, 64, qc, rcpB, atrsB)]
            for args in pending_post:
                norm_post(*args)
            if gen is not None:
                for _ in gen:
                    pass
            for st in range(4 * qc, 4 * qc + 4):
                enqueue_oproj(qc, st)
        while filler:
            filler.popleft()()

    nc.compile()
    return nc


def _host_tables():
    inv_freq = 1.0 / (10000.0 ** (np.arange(0, HD, 2, dtype=np.float32) / HD))
    pos = np.arange(S, dtype=np.float32)
    freqs = np.einsum('s,d->sd', pos, inv_freq)          # [S, 32]
    emb = np.concatenate([freqs, freqs], axis=-1)        # [S, 64]
    cosT = np.cos(emb).T.astype(np.float32)              # [64, S]
    sinT = np.sin(emb).T.astype(np.float32)
    c2 = np.ascontiguousarray(np.vstack([cosT, cosT]))   # [128, S]
    ss = np.ascontiguousarray(np.vstack([sinT, sinT]))
    # rotate-half as a matmul: out[d] = sum_d' R[d', d] * in[d']
    R64 = np.zeros((HD, HD), dtype=np.float32)
    for d in range(32):
        R64[d + 32, d] = -1.0       # out[d] = -in[d+32]
        R64[d, d + 32] = 1.0        # out[d+32] = in[d]
    rot = np.zeros((P, P), dtype=np.float32)
    rot[0:64, 0:64] = R64
    rot[64:128, 64:128] = R64
    # causal bias for a diagonal 128x128 tile in scores^T[k, q] layout
    kk = np.arange(P)[:, None]
    qq = np.arange(P)[None, :]
    msk = np.where(kk <= qq, 0.0, MASK_VAL).astype(np.float32)
    import ml_dtypes
    rot = rot.astype(ml_dtypes.bfloat16)   # exact: entries are 0/+-1
    return c2, ss, rot, msk


# q/o head order within a rank block: pair heads (u, u+4) in each 128-row tile
_HEAD_ORDER = [0, 4, 1, 5, 2, 6, 3, 7]


def _pack128(a):
    """[128*n, m] row-major -> [128, n*m] with block i at cols [m*i, m*(i+1))."""
    n = a.shape[0] // P
    return np.ascontiguousarray(
        a.reshape(n, P, a.shape[1]).transpose(1, 0, 2).reshape(P, -1))


def _make_in_maps(hidden_states, Wq, Wk, Wv, Wo):
    import ml_dtypes
    BF = ml_dtypes.bfloat16
    hs = np.asarray(hidden_states, dtype=np.float32)
    Wq = np.asarray(Wq, dtype=np.float32)
    Wk = np.asarray(Wk, dtype=np.float32)
    Wv = np.asarray(Wv, dtype=np.float32)
    Wo = np.asarray(Wo, dtype=np.float32)
    c2, ss, rot, msk = _host_tables()
    in_maps = []
    for c in range(8):
        b, r = c // 4, c % 4
        # row indices of Wq (= cols of Wo) for this rank, in device head order
        qrows = np.concatenate([
            np.arange(HD) + (NHL * r + u) * HD for u in _HEAD_ORDER
        ])
        xt_ = hs[b].T.astype(BF)                          # [H, S]
        # chunk-major packing: [128, sc, i, s']
        xp = xt_.reshape(NHT, P, NSC, QC).transpose(1, 2, 0, 3).reshape(P, -1)
        wq_ = _pack128(Wq[qrows, :].T.astype(BF))         # [128, 16*512]
        wk_ = Wk[NKO * r:NKO * (r + 1), :].T.astype(BF)   # [H, 128]
        wv_ = Wv[NKO * r:NKO * (r + 1), :].T.astype(BF)
        wkv_ = _pack128(np.concatenate([wk_, wv_], axis=1))
        wot_ = _pack128(Wo[:, qrows].T.astype(BF))        # [128, 4*2048]
        in_maps.append({
            "xall": np.ascontiguousarray(xp),
            "wqall": wq_,
            "wkv": wkv_,
            "wot": wot_,
            "c2": c2, "ss": ss, "msk": msk, "rot": rot,
        })
    return in_maps


_NC = None


def _get_nc():
    global _NC
    if _NC is None:
        _NC = build_nc()
    return _NC


def run_cores(hidden_states, Wq, Wk, Wv, Wo, **run_kwargs):
    """Run the SPMD kernel; returns (out [B,S,H] fp32, BassKernelResults)."""
    nc = _get_nc()
    in_maps = _make_in_maps(hidden_states, Wq, Wk, Wv, Wo)
    res = run_bass_kernel_spmd(nc, in_maps, list(range(8)), **run_kwargs)
    out = np.zeros((B, S, H), dtype=np.float32)
    for c in range(8):
        yb = np.asarray(res.results[c]["y"], dtype=np.float32)
        out[c // 4] += yb.reshape(P, NPT, S).transpose(1, 0, 2).reshape(S, H)
    return out, res


def kernel(hidden_states, Wq, Wk, Wv, Wo):
    out, _ = run_cores(hidden_states, Wq, Wk, Wv, Wo)
    return out


# revision 22
# speedup vs baseline: 1.3257x; 1.1176x over previous
"""Trainium2 Bass kernel for GQA attention (B=2, S=2048, H=2048, 32 Q heads,
8 KV heads, HD=64, RoPE, causal) with output projection.

Sharding: TP=4 over heads within each batch, DP=2 over batch -> 8 cores.
Core c handles batch c//4, head-rank c%4 (8 Q heads, 2 KV heads).
Each core computes a partial o_proj output [S, H]; the host sums the 4
partials per batch (cheaper than on-device all-reduce at these sizes).

v2: all inputs pre-cast to bf16 and pre-packed on host into [128, *]
contiguous layouts so every DMA is a plain 2D transfer straight into the
persistent SBUF tile (no staging, no on-device casts).  ACT runs exp and
copies only (one table set, zero reloads); the softmax reciprocal uses the
custom-DVE fast Newton-Raphson op.  AV matmuls and exp are causally
trimmed on diagonal tiles (diagonal-first ki order keeps PSUM has_written
coverage correct) -- no ep memsets.  o_proj accumulates into one
[128, 2048] bf16 tile per s-tile, stored with a single DMA; y is bf16
(host sums rank partials in fp32).

v3 (this file): software-pipelined emission.  The projection work for
chunk sc+1 is a generator of small quanta that are interleaved between
the attention steps of chunk sc, so the PE stays fed while ACT crunches
the exps (the attention inner loop is ACT-latency-bound).  Normalize is
split: the DVE part (reciprocal + copies) is emitted at the head-pair
boundary, the PE broadcast matmul + final multiply are deferred past the
next head-pair's first step so they never head-of-line-block the PE.

PSUM budget (8 banks):  scores 2x[128,1024] (4) | avpA/avpB [65,512] (2)
                        | proj/oproj/rope/V/rbc rotating [128,512] (2)
"""

import numpy as np
from contextlib import ExitStack

import concourse.bass as bass
import concourse.bacc as bacc
import concourse.mybir as mybir
import concourse.tile as tile
from concourse.bass_utils import run_bass_kernel_spmd

F32 = mybir.dt.float32
BF16 = mybir.dt.bfloat16
AF = mybir.ActivationFunctionType

B, S, H = 2, 2048, 2048
NH, NKV, HD = 32, 8, 64
TP = 4                      # head-parallel ranks per batch
NQO = NH // TP * HD         # 512 per-core q features (8 heads)
NKO = NKV // TP * HD        # 128 per-core kv features (2 heads)
NHL = NH // TP              # 8 local q heads
EXP_SCALE = 1.0 / 8.0       # 1/sqrt(HD)
MASK_VAL = -30000.0
P = 128
QC = 512                    # q-chunk (one PSUM bank of fp32)
NSC = S // QC               # 4 q/s chunks
NPT = S // P                # 16 partition tiles of S
NHT = H // P                # 16 partition tiles of H


def build_nc():
    nc = bacc.Bacc("TRN2", target_bir_lowering=False, debug=False, num_devices=8)

    xall = nc.dram_tensor("xall", [P, NSC * NHT * QC], BF16, kind="ExternalInput").ap()
    wqall = nc.dram_tensor("wqall", [P, NHT * NQO], BF16, kind="ExternalInput").ap()
    wkv = nc.dram_tensor("wkv", [P, NHT * 2 * NKO], BF16, kind="ExternalInput").ap()
    wot = nc.dram_tensor("wot", [P, 4 * S], BF16, kind="ExternalInput").ap()
    c2 = nc.dram_tensor("c2", [P, S], F32, kind="ExternalInput").ap()
    ss = nc.dram_tensor("ss", [P, S], F32, kind="ExternalInput").ap()
    msk = nc.dram_tensor("msk", [P, P], F32, kind="ExternalInput").ap()
    rot = nc.dram_tensor("rot", [P, P], BF16, kind="ExternalInput").ap()
    y = nc.dram_tensor("y", [P, NPT * S], BF16, kind="ExternalOutput").ap()

    with tile.TileContext(nc) as tc, ExitStack() as ctx:
        persist = ctx.enter_context(tc.tile_pool(name="persist", bufs=1))
        xpool = ctx.enter_context(tc.tile_pool(name="xpool", bufs=3))
        p1 = ctx.enter_context(tc.tile_pool(name="p1", bufs=2))
        p2 = ctx.enter_context(tc.tile_pool(name="p2", bufs=6))
        p2a = ctx.enter_context(tc.tile_pool(name="p2a", bufs=2))
        p3 = ctx.enter_context(tc.tile_pool(name="p3", bufs=2))
        psum = ctx.enter_context(tc.tile_pool(name="psum", bufs=2, space="PSUM"))

        # ---- persistent tiles ----
        c2_sb = persist.tile([P, S], F32, tag="c2", name="c2sb")
        ss_sb = persist.tile([P, S], F32, tag="ss", name="sssb")
        msk_sb = persist.tile([P, P], F32, tag="msk", name="msksb")
        rot_sb = persist.tile([P, P], BF16, tag="rot", name="rotsb")
        ones65b = persist.tile([65, 64], BF16, tag="ones65b", name="ones65b")

        wq_sb = persist.tile([P, NHT * NQO], BF16, tag="wq", name="wqsb")
        wkv_sb = persist.tile([P, NHT * 2 * NKO], BF16, tag="wkv", name="wkvsb")
        wot_sb = persist.tile([P, 4 * S], BF16, tag="wot", name="wotsb")

        qtbc = [[persist.tile([P, QC], BF16, tag=f"qtbc{t}_{sc}", name=f"qtbc{t}_{sc}")
                 for sc in range(NSC)] for t in range(4)]
        ktbc = [persist.tile([P, QC], BF16, tag=f"ktbc{sc}", name=f"ktbc{sc}")
                for sc in range(NSC)]
        vaug = [persist.tile([P, 130], BF16, tag=f"vaug{i}", name=f"vaug{i}")
                for i in range(NPT)]
        atbc = [[persist.tile([P, QC], BF16, tag=f"atbc{t}_{qc}", name=f"atbc{t}_{qc}")
                 for qc in range(NSC)] for t in range(4)]

        def wqt(i, t):         # Wq^T tile i, head-pair column block t
            return wq_sb[:, NQO * i + P * t: NQO * i + P * (t + 1)]

        def wkt(i):
            return wkv_sb[:, 2 * NKO * i: 2 * NKO * i + NKO]

        def wvt(i):
            return wkv_sb[:, 2 * NKO * i + NKO: 2 * NKO * (i + 1)]

        def wott(t, oc):       # Wo^T d-tile t, output H-chunk oc
            return wot_sb[:, S * t + QC * oc: S * t + QC * (oc + 1)]

        # ---- input DMAs: first-needed-first, round-robin the two queues ----
        xchunk = [None] * NSC
        qeng = [nc.sync, nc.gpsimd]

        def load_xchunk(sc, quarters=1):
            xc = xpool.tile([P, NHT * QC], BF16, tag="xchunk", name=f"xchunk{sc}")
            w = NHT * QC // quarters
            for qq in range(quarters):
                qeng[qq % 2].dma_start(
                    xc[:, w * qq: w * (qq + 1)],
                    xall[:, NHT * QC * sc + w * qq: NHT * QC * sc + w * (qq + 1)])
            xchunk[sc] = xc

        # interleave x-chunk-0 and wkv quarters so the first K chain can
        # start as soon as the first quarter lands (proj order is K,V,Q)
        xc0 = xpool.tile([P, NHT * QC], BF16, tag="xchunk", name="xchunk0")
        xchunk[0] = xc0
        x4 = NHT * QC // 4
        kv4 = NHT * 2 * NKO // 4
        for qq in range(4):
            nc.sync.dma_start(xc0[:, x4 * qq: x4 * (qq + 1)],
                              xall[:, x4 * qq: x4 * (qq + 1)])
            nc.gpsimd.dma_start(wkv_sb[:, kv4 * qq: kv4 * (qq + 1)],
                                wkv[:, kv4 * qq: kv4 * (qq + 1)])
        nc.sync.dma_start(rot_sb[:], rot[:])
        nc.gpsimd.dma_start(c2_sb[:], c2[:])
        nc.sync.dma_start(ss_sb[:], ss[:])
        wq2 = NHT * NQO // 2
        nc.gpsimd.dma_start(wq_sb[:, 0:wq2], wqall[:, 0:wq2])
        nc.sync.dma_start(wq_sb[:, wq2:], wqall[:, wq2:])
        nc.gpsimd.dma_start(msk_sb[:], msk[:])
        nc.gpsimd.memset(ones65b[64:65, :], 1.0)
        load_xchunk(1, quarters=2)
        nc.sync.dma_start(wot_sb[:, 0: 2 * S], wot[:, 0: 2 * S])
        nc.gpsimd.dma_start(wot_sb[:, 2 * S:], wot[:, 2 * S:])

        def xt(i, sc):
            return xchunk[sc][:, QC * i: QC * (i + 1)]

        def rope_tile(dst_ap, ps, sc):
            """RoPE: dst = raw*C2 + (R @ raw)*SS for one [128, 512] chunk."""
            ssl = slice(QC * sc, QC * (sc + 1))
            raw = p1.tile([P, QC], BF16, tag="rope_raw")
            nc.scalar.copy(raw[:], ps[:])
            rps = psum.tile([P, QC], F32, tag="pj", bufs=2, name="rps")
            nc.tensor.matmul(rps[:], lhsT=rot_sb[:], rhs=raw[:],
                             start=True, stop=True)
            t1 = p1.tile([P, QC], F32, tag="rope_t1")
            nc.gpsimd.tensor_mul(t1[:], raw[:], c2_sb[:, ssl])
            t2 = p1.tile([P, QC], F32, tag="rope_t2")
            nc.vector.tensor_mul(t2[:], rps[:], ss_sb[:, ssl])
            nc.vector.tensor_add(dst_ap, t1[:], t2[:])

        def proj_quanta(sc):
            """Generator: all projection work for chunk sc in small quanta.
            K and V first -- the next attention phase's first steps need
            ktbc/vaug; Q tiles t=1..3 are only needed hp steps later."""
            # K^T chain + rope
            ps = psum.tile([P, QC], F32, tag="pj", bufs=2, name="qkps")
            for i in range(NHT):
                nc.tensor.matmul(
                    ps[:], lhsT=wkt(i), rhs=xt(i, sc),
                    start=(i == 0), stop=(i == NHT - 1),
                )
                if i == 7:
                    yield
            yield
            rope_tile(ktbc[sc][:], ps, sc)
            yield
            # V tiles
            for j in range(4 * sc, 4 * sc + 4):
                jj = j - 4 * sc
                ps = psum.tile([P, NKO], F32, tag="pj", bufs=2, name="vps")
                for i in range(NHT):
                    nc.tensor.matmul(
                        ps[:], lhsT=xt(i, sc)[:, P * jj:P * (jj + 1)],
                        rhs=wvt(i),
                        start=(i == 0), stop=(i == NHT - 1),
                    )
                    if i == 7:
                        yield
                nc.vector.tensor_copy(vaug[j][:, 0:64], ps[:, 0:64])
                nc.vector.tensor_copy(vaug[j][:, 65:129], ps[:, 64:128])
                nc.gpsimd.memset(vaug[j][:, 64:65], 1.0)
                nc.gpsimd.memset(vaug[j][:, 129:130], 1.0)
                yield
            # Q^T chains + rope
            for t in range(4):
                ps = psum.tile([P, QC], F32, tag="pj", bufs=2, name="qkps")
                for i in range(NHT):
                    nc.tensor.matmul(
                        ps[:], lhsT=wqt(i, t), rhs=xt(i, sc),
                        start=(i == 0), stop=(i == NHT - 1),
                    )
                    if i == 7:
                        yield
                yield
                rope_tile(qtbc[t][sc][:], ps, sc)
                yield

        N_QUANTA = 23   # quanta per proj_quanta generator (4*3 + 3 + 4*2)

        def attn_step(hp, qc, ki, avpA, avpB, first, last):
            """One [128-k x 512-q] step for a head pair.  Diagonal tiles are
            causally trimmed: scores/exp/AV only touch cols >= col0."""
            j = ki - 4 * qc
            col0 = P * j if j >= 0 else 0
            kc = P * (ki % 4)
            sp = psum.tile([P, 2 * QC], F32, tag="sc", bufs=2, name="sp")
            nc.tensor.matmul(
                sp[:, col0:QC],
                lhsT=ktbc[ki // 4][0:64, kc:kc + P],
                rhs=qtbc[hp][qc][0:64, col0:QC],
                start=True, stop=True,
            )
            nc.tensor.matmul(
                sp[:, QC + col0:2 * QC],
                lhsT=ktbc[ki // 4][64:128, kc:kc + P],
                rhs=qtbc[hp][qc][64:128, col0:QC],
                start=True, stop=True,
            )
            ep = p2.tile([P, 2 * QC], BF16, tag="ep")
            if j >= 0:
                nc.vector.tensor_add(sp[:, col0:col0 + P],
                                     sp[:, col0:col0 + P], msk_sb[:])
                nc.vector.tensor_add(sp[:, QC + col0:QC + col0 + P],
                                     sp[:, QC + col0:QC + col0 + P], msk_sb[:])
                # one strided ACT op over both heads' trimmed column ranges
                spv = sp[:].rearrange("p (h w) -> p h w", h=2)
                epv = ep[:].rearrange("p (h w) -> p h w", h=2)
                nc.scalar.activation(epv[:, :, col0:QC], spv[:, :, col0:QC],
                                     AF.Exp, scale=EXP_SCALE)
            else:
                nc.scalar.activation(ep[:], sp[:], AF.Exp, scale=EXP_SCALE)
            nc.tensor.matmul(
                avpA[:, col0:QC], lhsT=vaug[ki][:, 0:65], rhs=ep[:, col0:QC],
                start=first, stop=last,
            )
            nc.tensor.matmul(
                avpB[:, col0:QC], lhsT=vaug[ki][:, 65:130],
                rhs=ep[:, QC + col0:2 * QC],
                start=first, stop=last,
            )

        def norm_copy(avp, copy_eng):
            """Copy AV+rowsum out of PSUM (frees the avp bank for the next
            group).  Engine split between ACT and DVE balances the queues."""
            av65 = p2a.tile([65, QC], F32, tag="av65")
            if copy_eng is nc.scalar:
                nc.scalar.copy(av65[:], avp[:])
            else:
                nc.vector.tensor_copy(av65[:], avp[:])
            return av65

        def norm_rcp(av65):
            """1/rowsum on DVE: classic magic-constant seed (0x7EF311C3 -
            bits(x), ~5% err) plus one Newton-Raphson step (~0.26% err,
            well under the bf16 quantum)."""
            y0 = p2a.tile([65, QC], F32, tag="y0")
            nc.vector.tensor_scalar(
                y0[64:65, :].bitcast(mybir.dt.int32),
                av65[64:65, :].bitcast(mybir.dt.int32),
                -1, 0x7EF311C3, op0=mybir.AluOpType.mult,
                op1=mybir.AluOpType.add)
            nrt = p2a.tile([65, QC], F32, tag="nrt")
            nc.vector.scalar_tensor_tensor(
                nrt[64:65, :], av65[64:65, :], -1.0, y0[64:65, :],
                op0=mybir.AluOpType.mult, op1=mybir.AluOpType.mult)
            rcp = p2a.tile([65, QC], BF16, tag="rcp")
            nc.vector.scalar_tensor_tensor(
                rcp[64:65, :], nrt[64:65, :], 2.0, y0[64:65, :],
                op0=mybir.AluOpType.add, op1=mybir.AluOpType.mult)
            return rcp

        def norm_post(hp, off, qc, rcp, av65):
            """PE broadcast of 1/rowsum + final normalize multiply."""
            rbc = psum.tile([64, QC], F32, tag="pj", bufs=2, name="rbc")
            nc.tensor.matmul(rbc[:], lhsT=ones65b[64:65, 0:64],
                             rhs=rcp[64:65, :], start=True, stop=True)
            nc.vector.tensor_mul(atbc[hp][qc][off:off + 64, :],
                                 av65[0:64, :], rbc[:])

        from collections import deque
        filler = deque()     # thunks of deferred PE-filler work (o_proj ocs)

        def enqueue_oproj(qc, st):
            stj = st - 4 * qc
            cell = {}

            def mk(oc):
                def th():
                    if 'ost' not in cell:
                        cell['ost'] = p3.tile([P, S], BF16, tag="ost",
                                              name=f"ost{st}")
                    ost = cell['ost']
                    op = psum.tile([P, QC], F32, tag="pj", bufs=2, name="op")
                    for ft in range(4):
                        nc.tensor.matmul(
                            op[:],
                            lhsT=atbc[ft][qc][:, P * stj:P * (stj + 1)],
                            rhs=wott(ft, oc),
                            start=(ft == 0), stop=(ft == 3),
                        )
                    if oc % 2 == 0:
                        nc.scalar.copy(ost[:, QC * oc:QC * (oc + 1)], op[:])
                    else:
                        nc.vector.tensor_copy(ost[:, QC * oc:QC * (oc + 1)], op[:])
                    if oc == NSC - 1:
                        nc.sync.dma_start(y[:, S * st:S * (st + 1)], ost[:])
                return th

            for oc in range(NSC):
                filler.append(mk(oc))

        # chunk 0 projections run alone (nothing to overlap with)
        for _ in proj_quanta(0):
            pass

        for sc in range(NSC):
            qc = sc
            if sc + 2 < NSC:
                load_xchunk(sc + 2, quarters=2)
            gen = proj_quanta(sc + 1) if sc + 1 < NSC else None
            rate = 2 if qc < 2 else 1
            pending = None   # (hp, av65A, av65B) awaiting rcp + norm_post
            rcps = None
            # maskless full-width off-diagonal steps first: the group-boundary
            # DVE work (copies, rcp chain) never blocks mask-adds -> exp; the
            # first step's start=True covers the full avp bank.
            nod = 4 * qc     # number of off-diagonal ki tiles
            lead = min(3, nod)
            ki_order = (list(range(lead)) + [4 * qc + j for j in range(4)]
                        + list(range(lead, nod)))
            for hp in range(4):
                avpA = psum.tile([65, QC], F32, tag="av", bufs=2, name="avpA")
                avpB = psum.tile([65, QC], F32, tag="av", bufs=2, name="avpB")
                for n, ki in enumerate(ki_order):
                    attn_step(hp, qc, ki, avpA, avpB,
                              n == 0, n == len(ki_order) - 1)
                    if n == 0 and pending is not None:
                        rcps = (norm_rcp(pending[1]), norm_rcp(pending[2]))
                    if n == 2 and pending is not None:
                        ph = pending[0]
                        norm_post(ph, 0, qc, rcps[0], pending[1])
                        norm_post(ph, 64, qc, rcps[1], pending[2])
                        pending = None
                    for _ in range(rate):
                        if gen is not None:
                            if next(gen, StopIteration) is StopIteration:
                                gen = None
                            else:
                                continue
                        if filler:
                            filler.popleft()()
                av65A = norm_copy(avpA, nc.scalar)
                av65B = norm_copy(avpB, nc.vector)
                pending = (hp, av65A, av65B)
            if pending is not None:
                rcps = (norm_rcp(pending[1]), norm_rcp(pending[2]))
                ph = pending[0]
                norm_post(ph, 0, qc, rcps[0], pending[1])
                norm_post(ph, 64, qc, rcps[1], pending[2])
            if gen is not None:
                for _ in gen:
                    pass
            for st in range(4 * qc, 4 * qc + 4):
                enqueue_oproj(qc, st)
        while filler:
            filler.popleft()()

    nc.compile()
    return nc


def _host_tables():
    inv_freq = 1.0 / (10000.0 ** (np.arange(0, HD, 2, dtype=np.float32) / HD))
    pos = np.arange(S, dtype=np.float32)
    freqs = np.einsum('s,d->sd', pos, inv_freq)          # [S, 32]
    emb = np.concatenate([freqs, freqs], axis=-1)        # [S, 64]
    cosT = np.cos(emb).T.astype(np.float32)              # [64, S]
    sinT = np.sin(emb).T.astype(np.float32)
    c2 = np.ascontiguousarray(np.vstack([cosT, cosT]))   # [128, S]
    ss = np.ascontiguousarray(np.vstack([sinT, sinT]))
    # rotate-half as a matmul: out[d] = sum_d' R[d', d] * in[d']
    R64 = np.zeros((HD, HD), dtype=np.float32)
    for d in range(32):
        R64[d + 32, d] = -1.0       # out[d] = -in[d+32]
        R64[d, d + 32] = 1.0        # out[d+32] = in[d]
    rot = np.zeros((P, P), dtype=np.float32)
    rot[0:64, 0:64] = R64
    rot[64:128, 64:128] = R64
    # causal bias for a diagonal 128x128 tile in scores^T[k, q] layout
    kk = np.arange(P)[:, None]
    qq = np.arange(P)[None, :]
    msk = np.where(kk <= qq, 0.0, MASK_VAL).astype(np.float32)
    import ml_dtypes
    rot = rot.astype(ml_dtypes.bfloat16)   # exact: entries are 0/+-1
    return c2, ss, rot, msk


# q/o head order within a rank block: pair heads (u, u+4) in each 128-row tile
_HEAD_ORDER = [0, 4, 1, 5, 2, 6, 3, 7]


def _pack128(a):
    """[128*n, m] row-major -> [128, n*m] with block i at cols [m*i, m*(i+1))."""
    n = a.shape[0] // P
    return np.ascontiguousarray(
        a.reshape(n, P, a.shape[1]).transpose(1, 0, 2).reshape(P, -1))


def _make_in_maps(hidden_states, Wq, Wk, Wv, Wo):
    import ml_dtypes
    BF = ml_dtypes.bfloat16
    hs = np.asarray(hidden_states, dtype=np.float32)
    Wq = np.asarray(Wq, dtype=np.float32)
    Wk = np.asarray(Wk, dtype=np.float32)
    Wv = np.asarray(Wv, dtype=np.float32)
    Wo = np.asarray(Wo, dtype=np.float32)
    c2, ss, rot, msk = _host_tables()
    in_maps = []
    for c in range(8):
        b, r = c // 4, c % 4
        # row indices of Wq (= cols of Wo) for this rank, in device head order
        qrows = np.concatenate([
            np.arange(HD) + (NHL * r + u) * HD for u in _HEAD_ORDER
        ])
        xt_ = hs[b].T.astype(BF)                          # [H, S]
        # chunk-major packing: [128, sc, i, s']
        xp = xt_.reshape(NHT, P, NSC, QC).transpose(1, 2, 0, 3).reshape(P, -1)
        wq_ = _pack128(Wq[qrows, :].T.astype(BF))         # [128, 16*512]
        wk_ = Wk[NKO * r:NKO * (r + 1), :].T.astype(BF)   # [H, 128]
        wv_ = Wv[NKO * r:NKO * (r + 1), :].T.astype(BF)
        wkv_ = _pack128(np.concatenate([wk_, wv_], axis=1))
        wot_ = _pack128(Wo[:, qrows].T.astype(BF))        # [128, 4*2048]
        in_maps.append({
            "xall": np.ascontiguousarray(xp),
            "wqall": wq_,
            "wkv": wkv_,
            "wot": wot_,
            "c2": c2, "ss": ss, "msk": msk, "rot": rot,
        })
    return in_maps


_NC = None


def _get_nc():
    global _NC
    if _NC is None:
        _NC = build_nc()
    return _NC


def run_cores(hidden_states, Wq, Wk, Wv, Wo, **run_kwargs):
    """Run the SPMD kernel; returns (out [B,S,H] fp32, BassKernelResults)."""
    nc = _get_nc()
    in_maps = _make_in_maps(hidden_states, Wq, Wk, Wv, Wo)
    res = run_bass_kernel_spmd(nc, in_maps, list(range(8)), **run_kwargs)
    out = np.zeros((B, S, H), dtype=np.float32)
    for c in range(8):
        yb = np.asarray(res.results[c]["y"], dtype=np.float32)
        out[c // 4] += yb.reshape(P, NPT, S).transpose(1, 0, 2).reshape(S, H)
    return out, res


def kernel(hidden_states, Wq, Wk, Wv, Wo):
    out, _ = run_cores(hidden_states, Wq, Wk, Wv, Wo)
    return out


# revision 25
# speedup vs baseline: 1.4183x; 1.0698x over previous
"""Trainium2 Bass kernel for GQA attention (B=2, S=2048, H=2048, 32 Q heads,
8 KV heads, HD=64, RoPE, causal) with output projection.

Sharding: TP=4 over heads within each batch, DP=2 over batch -> 8 cores.
Core c handles batch c//4, head-rank c%4 (8 Q heads, 2 KV heads).
Each core computes a partial o_proj output [S, H]; the host sums the 4
partials per batch (cheaper than on-device all-reduce at these sizes).

v2: all inputs pre-cast to bf16 and pre-packed on host into [128, *]
contiguous layouts so every DMA is a plain 2D transfer straight into the
persistent SBUF tile (no staging, no on-device casts).  ACT runs exp and
copies only (one table set, zero reloads); the softmax reciprocal uses the
custom-DVE fast Newton-Raphson op.  AV matmuls and exp are causally
trimmed on diagonal tiles (diagonal-first ki order keeps PSUM has_written
coverage correct) -- no ep memsets.  o_proj accumulates into one
[128, 2048] bf16 tile per s-tile, stored with a single DMA; y is bf16
(host sums rank partials in fp32).

v3 (this file): software-pipelined emission.  The projection work for
chunk sc+1 is a generator of small quanta that are interleaved between
the attention steps of chunk sc, so the PE stays fed while ACT crunches
the exps (the attention inner loop is ACT-latency-bound).  Normalize is
split: the DVE part (reciprocal + copies) is emitted at the head-pair
boundary, the PE broadcast matmul + final multiply are deferred past the
next head-pair's first step so they never head-of-line-block the PE.

PSUM budget (8 banks):  scores 2x[128,1024] (4) | avpA/avpB [65,512] (2)
                        | proj/oproj/rope/V/rbc rotating [128,512] (2)
"""

import numpy as np
from contextlib import ExitStack

import concourse.bass as bass
import concourse.bacc as bacc
import concourse.mybir as mybir
import concourse.tile as tile
from concourse.bass_utils import run_bass_kernel_spmd

F32 = mybir.dt.float32
BF16 = mybir.dt.bfloat16
AF = mybir.ActivationFunctionType

B, S, H = 2, 2048, 2048
NH, NKV, HD = 32, 8, 64
TP = 4                      # head-parallel ranks per batch
NQO = NH // TP * HD         # 512 per-core q features (8 heads)
NKO = NKV // TP * HD        # 128 per-core kv features (2 heads)
NHL = NH // TP              # 8 local q heads
EXP_SCALE = 1.0 / 8.0       # 1/sqrt(HD)
MASK_VAL = -30000.0
P = 128
QC = 512                    # q-chunk (one PSUM bank of fp32)
NSC = S // QC               # 4 q/s chunks
NPT = S // P                # 16 partition tiles of S
NHT = H // P                # 16 partition tiles of H


def build_nc():
    nc = bacc.Bacc("TRN2", target_bir_lowering=False, debug=False, num_devices=8)

    xall = nc.dram_tensor("xall", [P, NSC * NHT * QC], BF16, kind="ExternalInput").ap()
    wqall = nc.dram_tensor("wqall", [P, NHT * NQO], BF16, kind="ExternalInput").ap()
    wkv = nc.dram_tensor("wkv", [P, NHT * 2 * NKO], BF16, kind="ExternalInput").ap()
    wot = nc.dram_tensor("wot", [P, 4 * S], BF16, kind="ExternalInput").ap()
    c2 = nc.dram_tensor("c2", [P, S], F32, kind="ExternalInput").ap()
    ss = nc.dram_tensor("ss", [P, S], F32, kind="ExternalInput").ap()
    msk = nc.dram_tensor("msk", [P, P], F32, kind="ExternalInput").ap()
    rot = nc.dram_tensor("rot", [P, P], BF16, kind="ExternalInput").ap()
    y = nc.dram_tensor("y", [P, NPT * S], BF16, kind="ExternalOutput").ap()

    with tile.TileContext(nc) as tc, ExitStack() as ctx:
        persist = ctx.enter_context(tc.tile_pool(name="persist", bufs=1))
        xpool = ctx.enter_context(tc.tile_pool(name="xpool", bufs=3))
        p1 = ctx.enter_context(tc.tile_pool(name="p1", bufs=2))
        p2 = ctx.enter_context(tc.tile_pool(name="p2", bufs=6))
        p2a = ctx.enter_context(tc.tile_pool(name="p2a", bufs=2))
        p3 = ctx.enter_context(tc.tile_pool(name="p3", bufs=2))
        psum = ctx.enter_context(tc.tile_pool(name="psum", bufs=2, space="PSUM"))

        # ---- persistent tiles ----
        c2_sb = persist.tile([P, S], F32, tag="c2", name="c2sb")
        ss_sb = persist.tile([P, S], F32, tag="ss", name="sssb")
        msk_sb = persist.tile([P, P], F32, tag="msk", name="msksb")
        rot_sb = persist.tile([P, P], BF16, tag="rot", name="rotsb")
        ones65b = persist.tile([65, 64], BF16, tag="ones65b", name="ones65b")

        wq_sb = persist.tile([P, NHT * NQO], BF16, tag="wq", name="wqsb")
        wkv_sb = persist.tile([P, NHT * 2 * NKO], BF16, tag="wkv", name="wkvsb")
        wot_sb = persist.tile([P, 4 * S], BF16, tag="wot", name="wotsb")

        qtbc = [[persist.tile([P, QC], BF16, tag=f"qtbc{t}_{sc}", name=f"qtbc{t}_{sc}")
                 for sc in range(NSC)] for t in range(4)]
        ktbc = [persist.tile([P, QC], BF16, tag=f"ktbc{sc}", name=f"ktbc{sc}")
                for sc in range(NSC)]
        vaug = [persist.tile([P, 130], BF16, tag=f"vaug{i}", name=f"vaug{i}")
                for i in range(NPT)]
        atbc = [[persist.tile([P, QC], BF16, tag=f"atbc{t}_{qc}", name=f"atbc{t}_{qc}")
                 for qc in range(NSC)] for t in range(4)]

        def wqt(i, t):         # Wq^T tile i, head-pair column block t
            return wq_sb[:, NQO * i + P * t: NQO * i + P * (t + 1)]

        def wkt(i):
            return wkv_sb[:, 2 * NKO * i: 2 * NKO * i + NKO]

        def wvt(i):
            return wkv_sb[:, 2 * NKO * i + NKO: 2 * NKO * (i + 1)]

        def wott(t, oc):       # Wo^T d-tile t, output H-chunk oc
            return wot_sb[:, S * t + QC * oc: S * t + QC * (oc + 1)]

        # ---- input DMAs: first-needed-first, round-robin the two queues ----
        xchunk = [None] * NSC
        qeng = [nc.sync, nc.gpsimd]

        def load_xchunk(sc, quarters=1):
            xc = xpool.tile([P, NHT * QC], BF16, tag="xchunk", name=f"xchunk{sc}")
            w = NHT * QC // quarters
            for qq in range(quarters):
                qeng[qq % 2].dma_start(
                    xc[:, w * qq: w * (qq + 1)],
                    xall[:, NHT * QC * sc + w * qq: NHT * QC * sc + w * (qq + 1)])
            xchunk[sc] = xc

        # interleave x-chunk-0 and wkv quarters so the first K chain can
        # start as soon as the first quarter lands (proj order is K,V,Q)
        xc0 = xpool.tile([P, NHT * QC], BF16, tag="xchunk", name="xchunk0")
        xchunk[0] = xc0
        x4 = NHT * QC // 4
        kv4 = NHT * 2 * NKO // 4
        for qq in range(4):
            nc.sync.dma_start(xc0[:, x4 * qq: x4 * (qq + 1)],
                              xall[:, x4 * qq: x4 * (qq + 1)])
            nc.gpsimd.dma_start(wkv_sb[:, kv4 * qq: kv4 * (qq + 1)],
                                wkv[:, kv4 * qq: kv4 * (qq + 1)])
        nc.sync.dma_start(rot_sb[:], rot[:])
        nc.gpsimd.dma_start(c2_sb[:], c2[:])
        nc.sync.dma_start(ss_sb[:], ss[:])
        wq2 = NHT * NQO // 2
        nc.gpsimd.dma_start(wq_sb[:, 0:wq2], wqall[:, 0:wq2])
        nc.sync.dma_start(wq_sb[:, wq2:], wqall[:, wq2:])
        nc.gpsimd.dma_start(msk_sb[:], msk[:])
        nc.gpsimd.memset(ones65b[64:65, :], 1.0)
        load_xchunk(1, quarters=2)
        nc.sync.dma_start(wot_sb[:, 0: 2 * S], wot[:, 0: 2 * S])
        nc.gpsimd.dma_start(wot_sb[:, 2 * S:], wot[:, 2 * S:])

        def xt(i, sc):
            return xchunk[sc][:, QC * i: QC * (i + 1)]

        def rope_tile(dst_ap, ps, sc):
            """RoPE: dst = raw*C2 + (R @ raw)*SS for one [128, 512] chunk."""
            ssl = slice(QC * sc, QC * (sc + 1))
            raw = p1.tile([P, QC], BF16, tag="rope_raw")
            nc.scalar.copy(raw[:], ps[:])
            rps = psum.tile([P, QC], F32, tag="pj", bufs=2, name="rps")
            nc.tensor.matmul(rps[:], lhsT=rot_sb[:], rhs=raw[:],
                             start=True, stop=True)
            t1 = p1.tile([P, QC], F32, tag="rope_t1")
            nc.gpsimd.tensor_mul(t1[:], raw[:], c2_sb[:, ssl])
            t2 = p1.tile([P, QC], F32, tag="rope_t2")
            nc.vector.tensor_mul(t2[:], rps[:], ss_sb[:, ssl])
            nc.vector.tensor_add(dst_ap, t1[:], t2[:])

        def proj_quanta(sc):
            """Generator: all projection work for chunk sc in small quanta.
            K and V first -- the next attention phase's first steps need
            ktbc/vaug; Q tiles t=1..3 are only needed hp steps later."""
            # K^T chain + rope
            ps = psum.tile([P, QC], F32, tag="pj", bufs=2, name="qkps")
            for i in range(NHT):
                nc.tensor.matmul(
                    ps[:], lhsT=wkt(i), rhs=xt(i, sc),
                    start=(i == 0), stop=(i == NHT - 1),
                )
                if i == 7:
                    yield
            yield
            rope_tile(ktbc[sc][:], ps, sc)
            yield
            # V tiles
            for j in range(4 * sc, 4 * sc + 4):
                jj = j - 4 * sc
                ps = psum.tile([P, NKO], F32, tag="pj", bufs=2, name="vps")
                for i in range(NHT):
                    nc.tensor.matmul(
                        ps[:], lhsT=xt(i, sc)[:, P * jj:P * (jj + 1)],
                        rhs=wvt(i),
                        start=(i == 0), stop=(i == NHT - 1),
                    )
                    if i == 7:
                        yield
                nc.vector.tensor_copy(vaug[j][:, 0:64], ps[:, 0:64])
                nc.vector.tensor_copy(vaug[j][:, 65:129], ps[:, 64:128])
                nc.gpsimd.memset(vaug[j][:, 64:65], 1.0)
                nc.gpsimd.memset(vaug[j][:, 129:130], 1.0)
                yield
            # Q^T chains + rope
            for t in range(4):
                ps = psum.tile([P, QC], F32, tag="pj", bufs=2, name="qkps")
                for i in range(NHT):
                    nc.tensor.matmul(
                        ps[:], lhsT=wqt(i, t), rhs=xt(i, sc),
                        start=(i == 0), stop=(i == NHT - 1),
                    )
                    if i == 7:
                        yield
                yield
                rope_tile(qtbc[t][sc][:], ps, sc)
                yield

        N_QUANTA = 23   # quanta per proj_quanta generator (4*3 + 3 + 4*2)

        def attn_step(hp, qc, ki, avpA, avpB, first, last):
            """One [128-k x 512-q] step for a head pair.  Diagonal tiles are
            causally trimmed: scores/exp/AV only touch cols >= col0."""
            j = ki - 4 * qc
            col0 = P * j if j >= 0 else 0
            kc = P * (ki % 4)
            sp = psum.tile([P, 2 * QC], F32, tag="sc", bufs=2, name="sp")
            nc.tensor.matmul(
                sp[:, col0:QC],
                lhsT=ktbc[ki // 4][0:64, kc:kc + P],
                rhs=qtbc[hp][qc][0:64, col0:QC],
                start=True, stop=True,
            )
            nc.tensor.matmul(
                sp[:, QC + col0:2 * QC],
                lhsT=ktbc[ki // 4][64:128, kc:kc + P],
                rhs=qtbc[hp][qc][64:128, col0:QC],
                start=True, stop=True,
            )
            ep = p2.tile([P, 2 * QC], BF16, tag="ep")
            if j >= 0:
                nc.vector.tensor_add(sp[:, col0:col0 + P],
                                     sp[:, col0:col0 + P], msk_sb[:])
                nc.vector.tensor_add(sp[:, QC + col0:QC + col0 + P],
                                     sp[:, QC + col0:QC + col0 + P], msk_sb[:])
                # one strided ACT op over both heads' trimmed column ranges
                spv = sp[:].rearrange("p (h w) -> p h w", h=2)
                epv = ep[:].rearrange("p (h w) -> p h w", h=2)
                nc.scalar.activation(epv[:, :, col0:QC], spv[:, :, col0:QC],
                                     AF.Exp, scale=EXP_SCALE)
            else:
                nc.scalar.activation(ep[:], sp[:], AF.Exp, scale=EXP_SCALE)
            nc.tensor.matmul(
                avpA[:, col0:QC], lhsT=vaug[ki][:, 0:65], rhs=ep[:, col0:QC],
                start=first, stop=last,
            )
            nc.tensor.matmul(
                avpB[:, col0:QC], lhsT=vaug[ki][:, 65:130],
                rhs=ep[:, QC + col0:2 * QC],
                start=first, stop=last,
            )

        def norm_copy(avpA, avpB):
            """Copy both heads' AV+rowsum out of PSUM into one paired tile
            (frees the avp banks).  Engine split ACT/DVE balances queues."""
            apair = p2a.tile([65, 2 * QC], F32, tag="apair")
            nc.scalar.copy(apair[:, 0:QC], avpA[:])
            nc.vector.tensor_copy(apair[:, QC:2 * QC], avpB[:])
            return apair

        def norm_rcp(apair):
            """1/rowsum for both heads in one DVE pass: classic magic-constant
            seed (0x7EF311C3 - bits(x), ~5% err) plus one Newton-Raphson step
            (~0.26% err, well under the bf16 quantum)."""
            y0 = p2a.tile([65, 2 * QC], F32, tag="y0")
            nc.vector.tensor_scalar(
                y0[64:65, :].bitcast(mybir.dt.int32),
                apair[64:65, :].bitcast(mybir.dt.int32),
                -1, 0x7EF311C3, op0=mybir.AluOpType.mult,
                op1=mybir.AluOpType.add)
            nrt = p2a.tile([65, 2 * QC], F32, tag="nrt")
            nc.vector.scalar_tensor_tensor(
                nrt[64:65, :], apair[64:65, :], -1.0, y0[64:65, :],
                op0=mybir.AluOpType.mult, op1=mybir.AluOpType.mult)
            rcp = p2a.tile([65, 2 * QC], BF16, tag="rcp")
            nc.vector.scalar_tensor_tensor(
                rcp[64:65, :], nrt[64:65, :], 2.0, y0[64:65, :],
                op0=mybir.AluOpType.add, op1=mybir.AluOpType.mult)
            return rcp

        def norm_post(hp, off, qc, rcp, apair):
            """PE broadcast of 1/rowsum + final normalize multiply."""
            half = slice(0, QC) if off == 0 else slice(QC, 2 * QC)
            rbc = psum.tile([64, QC], F32, tag="pj", bufs=2, name="rbc")
            nc.tensor.matmul(rbc[:], lhsT=ones65b[64:65, 0:64],
                             rhs=rcp[64:65, half], start=True, stop=True)
            nc.vector.tensor_mul(atbc[hp][qc][off:off + 64, :],
                                 apair[0:64, half], rbc[:])

        from collections import deque
        filler = deque()     # thunks of deferred PE-filler work (o_proj ocs)

        def enqueue_oproj(qc, st):
            stj = st - 4 * qc
            cell = {}

            def mk(oc):
                def th():
                    if 'ost' not in cell:
                        cell['ost'] = p3.tile([P, S], BF16, tag="ost",
                                              name=f"ost{st}")
                    ost = cell['ost']
                    op = psum.tile([P, QC], F32, tag="pj", bufs=2, name="op")
                    for ft in range(4):
                        nc.tensor.matmul(
                            op[:],
                            lhsT=atbc[ft][qc][:, P * stj:P * (stj + 1)],
                            rhs=wott(ft, oc),
                            start=(ft == 0), stop=(ft == 3),
                        )
                    if oc % 2 == 0:
                        nc.scalar.copy(ost[:, QC * oc:QC * (oc + 1)], op[:])
                    else:
                        nc.vector.tensor_copy(ost[:, QC * oc:QC * (oc + 1)], op[:])
                    if oc == NSC - 1:
                        nc.sync.dma_start(y[:, S * st:S * (st + 1)], ost[:])
                return th

            for oc in range(NSC):
                filler.append(mk(oc))

        # chunk 0 projections run alone (nothing to overlap with)
        for _ in proj_quanta(0):
            pass

        for sc in range(NSC):
            qc = sc
            if sc + 2 < NSC:
                load_xchunk(sc + 2, quarters=2)
            gen = proj_quanta(sc + 1) if sc + 1 < NSC else None
            pending = None   # (hp, apair) awaiting rcp + norm_post
            rcps = None
            # spread the filler work evenly over this phase's steps
            total_steps = 4 * (4 * qc + 4)
            est = len(filler) + (N_QUANTA if gen is not None else 0)
            pump_rate = est / total_steps
            acc = 0.0
            # maskless full-width off-diagonal steps first: the group-boundary
            # DVE work (copies, rcp chain) never blocks mask-adds -> exp; the
            # first step's start=True covers the full avp bank.
            nod = 4 * qc     # number of off-diagonal ki tiles
            lead = min(3, nod)
            ki_order = (list(range(lead)) + [4 * qc + j for j in range(4)]
                        + list(range(lead, nod)))
            for hp in range(4):
                avpA = psum.tile([65, QC], F32, tag="av", bufs=2, name="avpA")
                avpB = psum.tile([65, QC], F32, tag="av", bufs=2, name="avpB")
                for n, ki in enumerate(ki_order):
                    attn_step(hp, qc, ki, avpA, avpB,
                              n == 0, n == len(ki_order) - 1)
                    if n == 0 and pending is not None:
                        rcps = norm_rcp(pending[1])
                    if n == 2 and pending is not None:
                        ph = pending[0]
                        norm_post(ph, 0, qc, rcps, pending[1])
                        norm_post(ph, 64, qc, rcps, pending[1])
                        pending = None
                    acc += pump_rate
                    while acc >= 1.0:
                        acc -= 1.0
                        if gen is not None:
                            if next(gen, StopIteration) is StopIteration:
                                gen = None
                            else:
                                continue
                        if filler:
                            filler.popleft()()
                pending = (hp, norm_copy(avpA, avpB))
            if pending is not None:
                rcps = norm_rcp(pending[1])
                ph = pending[0]
                norm_post(ph, 0, qc, rcps, pending[1])
                norm_post(ph, 64, qc, rcps, pending[1])
            if gen is not None:
                for _ in gen:
                    pass
            for st in range(4 * qc, 4 * qc + 4):
                enqueue_oproj(qc, st)
        while filler:
            filler.popleft()()

    nc.compile()
    return nc


def _host_tables():
    inv_freq = 1.0 / (10000.0 ** (np.arange(0, HD, 2, dtype=np.float32) / HD))
    pos = np.arange(S, dtype=np.float32)
    freqs = np.einsum('s,d->sd', pos, inv_freq)          # [S, 32]
    emb = np.concatenate([freqs, freqs], axis=-1)        # [S, 64]
    cosT = np.cos(emb).T.astype(np.float32)              # [64, S]
    sinT = np.sin(emb).T.astype(np.float32)
    c2 = np.ascontiguousarray(np.vstack([cosT, cosT]))   # [128, S]
    ss = np.ascontiguousarray(np.vstack([sinT, sinT]))
    # rotate-half as a matmul: out[d] = sum_d' R[d', d] * in[d']
    R64 = np.zeros((HD, HD), dtype=np.float32)
    for d in range(32):
        R64[d + 32, d] = -1.0       # out[d] = -in[d+32]
        R64[d, d + 32] = 1.0        # out[d+32] = in[d]
    rot = np.zeros((P, P), dtype=np.float32)
    rot[0:64, 0:64] = R64
    rot[64:128, 64:128] = R64
    # causal bias for a diagonal 128x128 tile in scores^T[k, q] layout
    kk = np.arange(P)[:, None]
    qq = np.arange(P)[None, :]
    msk = np.where(kk <= qq, 0.0, MASK_VAL).astype(np.float32)
    import ml_dtypes
    rot = rot.astype(ml_dtypes.bfloat16)   # exact: entries are 0/+-1
    return c2, ss, rot, msk


# q/o head order within a rank block: pair heads (u, u+4) in each 128-row tile
_HEAD_ORDER = [0, 4, 1, 5, 2, 6, 3, 7]


def _pack128(a):
    """[128*n, m] row-major -> [128, n*m] with block i at cols [m*i, m*(i+1))."""
    n = a.shape[0] // P
    return np.ascontiguousarray(
        a.reshape(n, P, a.shape[1]).transpose(1, 0, 2).reshape(P, -1))


def _make_in_maps(hidden_states, Wq, Wk, Wv, Wo):
    import ml_dtypes
    BF = ml_dtypes.bfloat16
    hs = np.asarray(hidden_states, dtype=np.float32)
    Wq = np.asarray(Wq, dtype=np.float32)
    Wk = np.asarray(Wk, dtype=np.float32)
    Wv = np.asarray(Wv, dtype=np.float32)
    Wo = np.asarray(Wo, dtype=np.float32)
    c2, ss, rot, msk = _host_tables()
    in_maps = []
    for c in range(8):
        b, r = c // 4, c % 4
        # row indices of Wq (= cols of Wo) for this rank, in device head order
        qrows = np.concatenate([
            np.arange(HD) + (NHL * r + u) * HD for u in _HEAD_ORDER
        ])
        xt_ = hs[b].T.astype(BF)                          # [H, S]
        # chunk-major packing: [128, sc, i, s']
        xp = xt_.reshape(NHT, P, NSC, QC).transpose(1, 2, 0, 3).reshape(P, -1)
        wq_ = _pack128(Wq[qrows, :].T.astype(BF))         # [128, 16*512]
        wk_ = Wk[NKO * r:NKO * (r + 1), :].T.astype(BF)   # [H, 128]
        wv_ = Wv[NKO * r:NKO * (r + 1), :].T.astype(BF)
        wkv_ = _pack128(np.concatenate([wk_, wv_], axis=1))
        wot_ = _pack128(Wo[:, qrows].T.astype(BF))        # [128, 4*2048]
        in_maps.append({
            "xall": np.ascontiguousarray(xp),
            "wqall": wq_,
            "wkv": wkv_,
            "wot": wot_,
            "c2": c2, "ss": ss, "msk": msk, "rot": rot,
        })
    return in_maps


_NC = None


def _get_nc():
    global _NC
    if _NC is None:
        _NC = build_nc()
    return _NC


def run_cores(hidden_states, Wq, Wk, Wv, Wo, **run_kwargs):
    """Run the SPMD kernel; returns (out [B,S,H] fp32, BassKernelResults)."""
    nc = _get_nc()
    in_maps = _make_in_maps(hidden_states, Wq, Wk, Wv, Wo)
    res = run_bass_kernel_spmd(nc, in_maps, list(range(8)), **run_kwargs)
    out = np.zeros((B, S, H), dtype=np.float32)
    for c in range(8):
        yb = np.asarray(res.results[c]["y"], dtype=np.float32)
        out[c // 4] += yb.reshape(P, NPT, S).transpose(1, 0, 2).reshape(S, H)
    return out, res


def kernel(hidden_states, Wq, Wk, Wv, Wo):
    out, _ = run_cores(hidden_states, Wq, Wk, Wv, Wo)
    return out


# revision 27
# speedup vs baseline: 1.4453x; 1.0190x over previous
"""Trainium2 Bass kernel for GQA attention (B=2, S=2048, H=2048, 32 Q heads,
8 KV heads, HD=64, RoPE, causal) with output projection.

Sharding: TP=4 over heads within each batch, DP=2 over batch -> 8 cores.
Core c handles batch c//4, head-rank c%4 (8 Q heads, 2 KV heads).
Each core computes a partial o_proj output [S, H]; the host sums the 4
partials per batch (cheaper than on-device all-reduce at these sizes).

v2: all inputs pre-cast to bf16 and pre-packed on host into [128, *]
contiguous layouts so every DMA is a plain 2D transfer straight into the
persistent SBUF tile (no staging, no on-device casts).  ACT runs exp and
copies only (one table set, zero reloads); the softmax reciprocal uses the
custom-DVE fast Newton-Raphson op.  AV matmuls and exp are causally
trimmed on diagonal tiles (diagonal-first ki order keeps PSUM has_written
coverage correct) -- no ep memsets.  o_proj accumulates into one
[128, 2048] bf16 tile per s-tile, stored with a single DMA; y is bf16
(host sums rank partials in fp32).

v3 (this file): software-pipelined emission.  The projection work for
chunk sc+1 is a generator of small quanta that are interleaved between
the attention steps of chunk sc, so the PE stays fed while ACT crunches
the exps (the attention inner loop is ACT-latency-bound).  Normalize is
split: the DVE part (reciprocal + copies) is emitted at the head-pair
boundary, the PE broadcast matmul + final multiply are deferred past the
next head-pair's first step so they never head-of-line-block the PE.

PSUM budget (8 banks):  scores 2x[128,1024] (4) | avpA/avpB [65,512] (2)
                        | proj/oproj/rope/V/rbc rotating [128,512] (2)
"""

import numpy as np
from contextlib import ExitStack

import concourse.bass as bass
import concourse.bacc as bacc
import concourse.mybir as mybir
import concourse.tile as tile
from concourse.bass_utils import run_bass_kernel_spmd

F32 = mybir.dt.float32
BF16 = mybir.dt.bfloat16
AF = mybir.ActivationFunctionType

B, S, H = 2, 2048, 2048
NH, NKV, HD = 32, 8, 64
TP = 4                      # head-parallel ranks per batch
NQO = NH // TP * HD         # 512 per-core q features (8 heads)
NKO = NKV // TP * HD        # 128 per-core kv features (2 heads)
NHL = NH // TP              # 8 local q heads
EXP_SCALE = 1.0 / 8.0       # 1/sqrt(HD)
MASK_VAL = -30000.0
P = 128
QC = 512                    # q-chunk (one PSUM bank of fp32)
NSC = S // QC               # 4 q/s chunks
NPT = S // P                # 16 partition tiles of S
NHT = H // P                # 16 partition tiles of H


def build_nc():
    nc = bacc.Bacc("TRN2", target_bir_lowering=False, debug=False, num_devices=8)

    xall = nc.dram_tensor("xall", [P, NSC * NHT * QC], BF16, kind="ExternalInput").ap()
    wqall = nc.dram_tensor("wqall", [P, NHT * NQO], BF16, kind="ExternalInput").ap()
    wkv = nc.dram_tensor("wkv", [P, NHT * 2 * NKO], BF16, kind="ExternalInput").ap()
    wot = nc.dram_tensor("wot", [P, 4 * S], BF16, kind="ExternalInput").ap()
    c2 = nc.dram_tensor("c2", [P, S], F32, kind="ExternalInput").ap()
    ss = nc.dram_tensor("ss", [P, S], F32, kind="ExternalInput").ap()
    msk = nc.dram_tensor("msk", [P, P], F32, kind="ExternalInput").ap()
    rot = nc.dram_tensor("rot", [P, P], BF16, kind="ExternalInput").ap()
    y = nc.dram_tensor("y", [P, NPT * S], BF16, kind="ExternalOutput").ap()

    with tile.TileContext(nc) as tc, ExitStack() as ctx:
        persist = ctx.enter_context(tc.tile_pool(name="persist", bufs=1))
        xpool = ctx.enter_context(tc.tile_pool(name="xpool", bufs=3))
        p1 = ctx.enter_context(tc.tile_pool(name="p1", bufs=2))
        p2 = ctx.enter_context(tc.tile_pool(name="p2", bufs=6))
        p2a = ctx.enter_context(tc.tile_pool(name="p2a", bufs=2))
        p3 = ctx.enter_context(tc.tile_pool(name="p3", bufs=2))
        psum = ctx.enter_context(tc.tile_pool(name="psum", bufs=2, space="PSUM"))

        # ---- persistent tiles ----
        c2_sb = persist.tile([P, S], F32, tag="c2", name="c2sb")
        ss_sb = persist.tile([P, S], F32, tag="ss", name="sssb")
        msk_sb = persist.tile([P, P], F32, tag="msk", name="msksb")
        rot_sb = persist.tile([P, P], BF16, tag="rot", name="rotsb")
        ones65b = persist.tile([65, 64], BF16, tag="ones65b", name="ones65b")

        wq_sb = persist.tile([P, NHT * NQO], BF16, tag="wq", name="wqsb")
        wkv_sb = persist.tile([P, NHT * 2 * NKO], BF16, tag="wkv", name="wkvsb")
        wot_sb = persist.tile([P, 4 * S], BF16, tag="wot", name="wotsb")

        qtbc = [[persist.tile([P, QC], BF16, tag=f"qtbc{t}_{sc}", name=f"qtbc{t}_{sc}")
                 for sc in range(NSC)] for t in range(4)]
        ktbc = [persist.tile([P, QC], BF16, tag=f"ktbc{sc}", name=f"ktbc{sc}")
                for sc in range(NSC)]
        vaug = [persist.tile([P, 130], BF16, tag=f"vaug{i}", name=f"vaug{i}")
                for i in range(NPT)]
        atbc = [[persist.tile([P, QC], BF16, tag=f"atbc{t}_{qc}", name=f"atbc{t}_{qc}")
                 for qc in range(NSC)] for t in range(4)]

        def wqt(i, t):         # Wq^T tile i, head-pair column block t
            return wq_sb[:, NQO * i + P * t: NQO * i + P * (t + 1)]

        def wkt(i):
            return wkv_sb[:, 2 * NKO * i: 2 * NKO * i + NKO]

        def wvt(i):
            return wkv_sb[:, 2 * NKO * i + NKO: 2 * NKO * (i + 1)]

        def wott(t, oc):       # Wo^T d-tile t, output H-chunk oc
            return wot_sb[:, S * t + QC * oc: S * t + QC * (oc + 1)]

        # ---- input DMAs: first-needed-first, round-robin the two queues ----
        xchunk = [None] * NSC
        qeng = [nc.sync, nc.gpsimd]

        def load_xchunk(sc, quarters=1):
            xc = xpool.tile([P, NHT * QC], BF16, tag="xchunk", name=f"xchunk{sc}")
            w = NHT * QC // quarters
            for qq in range(quarters):
                qeng[qq % 2].dma_start(
                    xc[:, w * qq: w * (qq + 1)],
                    xall[:, NHT * QC * sc + w * qq: NHT * QC * sc + w * (qq + 1)])
            xchunk[sc] = xc

        # interleave x-chunk-0 and wkv quarters so the first K chain can
        # start as soon as the first quarter lands (proj order is K,V,Q)
        xc0 = xpool.tile([P, NHT * QC], BF16, tag="xchunk", name="xchunk0")
        xchunk[0] = xc0
        x4 = NHT * QC // 4
        kv4 = NHT * 2 * NKO // 4
        for qq in range(4):
            nc.sync.dma_start(xc0[:, x4 * qq: x4 * (qq + 1)],
                              xall[:, x4 * qq: x4 * (qq + 1)])
            nc.gpsimd.dma_start(wkv_sb[:, kv4 * qq: kv4 * (qq + 1)],
                                wkv[:, kv4 * qq: kv4 * (qq + 1)])
        nc.sync.dma_start(rot_sb[:], rot[:])
        nc.gpsimd.dma_start(c2_sb[:], c2[:])
        nc.sync.dma_start(ss_sb[:], ss[:])
        wq2 = NHT * NQO // 2
        nc.gpsimd.dma_start(wq_sb[:, 0:wq2], wqall[:, 0:wq2])
        nc.sync.dma_start(wq_sb[:, wq2:], wqall[:, wq2:])
        nc.gpsimd.dma_start(msk_sb[:], msk[:])
        nc.gpsimd.memset(ones65b[64:65, :], 1.0)
        load_xchunk(1, quarters=2)
        nc.sync.dma_start(wot_sb[:, 0: 2 * S], wot[:, 0: 2 * S])
        nc.gpsimd.dma_start(wot_sb[:, 2 * S:], wot[:, 2 * S:])

        def xt(i, sc):
            return xchunk[sc][:, QC * i: QC * (i + 1)]

        def rope_tile(dst_ap, ps, sc):
            """RoPE: dst = raw*C2 + (R @ raw)*SS for one [128, 512] chunk."""
            ssl = slice(QC * sc, QC * (sc + 1))
            raw = p1.tile([P, QC], BF16, tag="rope_raw")
            nc.scalar.copy(raw[:], ps[:])
            rps = psum.tile([P, QC], F32, tag="pj", bufs=2, name="rps")
            nc.tensor.matmul(rps[:], lhsT=rot_sb[:], rhs=raw[:],
                             start=True, stop=True)
            t1 = p1.tile([P, QC], F32, tag="rope_t1")
            nc.gpsimd.tensor_mul(t1[:], raw[:], c2_sb[:, ssl])
            t2 = p1.tile([P, QC], F32, tag="rope_t2")
            nc.vector.tensor_mul(t2[:], rps[:], ss_sb[:, ssl])
            nc.gpsimd.tensor_add(dst_ap, t1[:], t2[:])

        def proj_quanta(sc):
            """Generator: all projection work for chunk sc in small quanta.
            K and V first -- the next attention phase's first steps need
            ktbc/vaug; Q tiles t=1..3 are only needed hp steps later."""
            # K^T chain + rope
            ps = psum.tile([P, QC], F32, tag="pj", bufs=2, name="qkps")
            for i in range(NHT):
                nc.tensor.matmul(
                    ps[:], lhsT=wkt(i), rhs=xt(i, sc),
                    start=(i == 0), stop=(i == NHT - 1),
                )
                if i == 7:
                    yield
            yield
            rope_tile(ktbc[sc][:], ps, sc)
            yield
            # V tiles
            for j in range(4 * sc, 4 * sc + 4):
                jj = j - 4 * sc
                ps = psum.tile([P, NKO], F32, tag="pj", bufs=2, name="vps")
                for i in range(NHT):
                    nc.tensor.matmul(
                        ps[:], lhsT=xt(i, sc)[:, P * jj:P * (jj + 1)],
                        rhs=wvt(i),
                        start=(i == 0), stop=(i == NHT - 1),
                    )
                    if i == 7:
                        yield
                nc.vector.tensor_copy(vaug[j][:, 0:64], ps[:, 0:64])
                nc.vector.tensor_copy(vaug[j][:, 65:129], ps[:, 64:128])
                nc.gpsimd.memset(vaug[j][:, 64:65], 1.0)
                nc.gpsimd.memset(vaug[j][:, 129:130], 1.0)
                yield
            # Q^T chains + rope
            for t in range(4):
                ps = psum.tile([P, QC], F32, tag="pj", bufs=2, name="qkps")
                for i in range(NHT):
                    nc.tensor.matmul(
                        ps[:], lhsT=wqt(i, t), rhs=xt(i, sc),
                        start=(i == 0), stop=(i == NHT - 1),
                    )
                    if i == 7:
                        yield
                yield
                rope_tile(qtbc[t][sc][:], ps, sc)
                yield

        N_QUANTA = 23   # quanta per proj_quanta generator (4*3 + 3 + 4*2)

        def attn_step(hp, qc, ki, avpA, avpB, first, last):
            """One [128-k x 512-q] step for a head pair.  Diagonal tiles are
            causally trimmed: scores/exp/AV only touch cols >= col0."""
            j = ki - 4 * qc
            col0 = P * j if j >= 0 else 0
            kc = P * (ki % 4)
            sp = psum.tile([P, 2 * QC], F32, tag="sc", bufs=2, name="sp")
            nc.tensor.matmul(
                sp[:, col0:QC],
                lhsT=ktbc[ki // 4][0:64, kc:kc + P],
                rhs=qtbc[hp][qc][0:64, col0:QC],
                start=True, stop=True,
            )
            nc.tensor.matmul(
                sp[:, QC + col0:2 * QC],
                lhsT=ktbc[ki // 4][64:128, kc:kc + P],
                rhs=qtbc[hp][qc][64:128, col0:QC],
                start=True, stop=True,
            )
            ep = p2.tile([P, 2 * QC], BF16, tag="ep")
            if j >= 0:
                nc.vector.tensor_add(sp[:, col0:col0 + P],
                                     sp[:, col0:col0 + P], msk_sb[:])
                nc.vector.tensor_add(sp[:, QC + col0:QC + col0 + P],
                                     sp[:, QC + col0:QC + col0 + P], msk_sb[:])
                # one strided ACT op over both heads' trimmed column ranges
                spv = sp[:].rearrange("p (h w) -> p h w", h=2)
                epv = ep[:].rearrange("p (h w) -> p h w", h=2)
                nc.scalar.activation(epv[:, :, col0:QC], spv[:, :, col0:QC],
                                     AF.Exp, scale=EXP_SCALE)
            else:
                nc.scalar.activation(ep[:], sp[:], AF.Exp, scale=EXP_SCALE)
            nc.tensor.matmul(
                avpA[:, col0:QC], lhsT=vaug[ki][:, 0:65], rhs=ep[:, col0:QC],
                start=first, stop=last,
            )
            nc.tensor.matmul(
                avpB[:, col0:QC], lhsT=vaug[ki][:, 65:130],
                rhs=ep[:, QC + col0:2 * QC],
                start=first, stop=last,
            )

        def norm_copy(avpA, avpB):
            """Copy both heads' AV+rowsum out of PSUM into one paired tile
            (frees the avp banks).  Engine split ACT/DVE balances queues."""
            apair = p2a.tile([65, 2 * QC], F32, tag="apair")
            nc.scalar.copy(apair[:, 0:QC], avpA[:])
            nc.vector.tensor_copy(apair[:, QC:2 * QC], avpB[:])
            return apair

        def norm_rcp(apair):
            """1/rowsum for both heads in one DVE pass: classic magic-constant
            seed (0x7EF311C3 - bits(x), ~5% err) plus one Newton-Raphson step
            (~0.26% err, well under the bf16 quantum)."""
            y0 = p2a.tile([65, 2 * QC], F32, tag="y0")
            nc.vector.tensor_scalar(
                y0[64:65, :].bitcast(mybir.dt.int32),
                apair[64:65, :].bitcast(mybir.dt.int32),
                -1, 0x7EF311C3, op0=mybir.AluOpType.mult,
                op1=mybir.AluOpType.add)
            nrt = p2a.tile([65, 2 * QC], F32, tag="nrt")
            nc.vector.scalar_tensor_tensor(
                nrt[64:65, :], apair[64:65, :], -1.0, y0[64:65, :],
                op0=mybir.AluOpType.mult, op1=mybir.AluOpType.mult)
            rcp = p2a.tile([65, 2 * QC], BF16, tag="rcp")
            nc.vector.scalar_tensor_tensor(
                rcp[64:65, :], nrt[64:65, :], 2.0, y0[64:65, :],
                op0=mybir.AluOpType.add, op1=mybir.AluOpType.mult)
            return rcp

        def norm_post(hp, off, qc, rcp, apair):
            """PE broadcast of 1/rowsum + final normalize multiply."""
            half = slice(0, QC) if off == 0 else slice(QC, 2 * QC)
            rbc = psum.tile([64, QC], F32, tag="pj", bufs=2, name="rbc")
            nc.tensor.matmul(rbc[:], lhsT=ones65b[64:65, 0:64],
                             rhs=rcp[64:65, half], start=True, stop=True)
            nc.vector.tensor_mul(atbc[hp][qc][off:off + 64, :],
                                 apair[0:64, half], rbc[:])

        from collections import deque
        filler = deque()     # thunks of deferred PE-filler work (o_proj ocs)

        def enqueue_oproj(qc, st):
            stj = st - 4 * qc
            cell = {}

            def mk(oc):
                def th():
                    if 'ost' not in cell:
                        cell['ost'] = p3.tile([P, S], BF16, tag="ost",
                                              name=f"ost{st}")
                    ost = cell['ost']
                    op = psum.tile([P, QC], F32, tag="pj", bufs=2, name="op")
                    for ft in range(4):
                        nc.tensor.matmul(
                            op[:],
                            lhsT=atbc[ft][qc][:, P * stj:P * (stj + 1)],
                            rhs=wott(ft, oc),
                            start=(ft == 0), stop=(ft == 3),
                        )
                    if oc % 2 == 0:
                        nc.scalar.copy(ost[:, QC * oc:QC * (oc + 1)], op[:])
                    else:
                        nc.vector.tensor_copy(ost[:, QC * oc:QC * (oc + 1)], op[:])
                    if oc == NSC - 1:
                        nc.sync.dma_start(y[:, S * st:S * (st + 1)], ost[:])
                return th

            for oc in range(NSC):
                filler.append(mk(oc))

        # chunk 0 projections run alone (nothing to overlap with)
        for _ in proj_quanta(0):
            pass

        for sc in range(NSC):
            qc = sc
            if sc + 2 < NSC:
                load_xchunk(sc + 2, quarters=2)
            gen = proj_quanta(sc + 1) if sc + 1 < NSC else None
            pending = None   # (hp, apair) awaiting rcp + norm_post
            rcps = None
            # spread the filler work evenly over this phase's steps
            total_steps = 4 * (4 * qc + 4)
            est = len(filler) + (N_QUANTA if gen is not None else 0)
            pump_rate = est / total_steps
            acc = 0.0
            # maskless full-width off-diagonal steps first: the group-boundary
            # DVE work (copies, rcp chain) never blocks mask-adds -> exp; the
            # first step's start=True covers the full avp bank.
            nod = 4 * qc     # number of off-diagonal ki tiles
            lead = min(3, nod)
            ki_order = (list(range(lead)) + [4 * qc + j for j in range(4)]
                        + list(range(lead, nod)))
            for hp in range(4):
                avpA = psum.tile([65, QC], F32, tag="av", bufs=2, name="avpA")
                avpB = psum.tile([65, QC], F32, tag="av", bufs=2, name="avpB")
                for n, ki in enumerate(ki_order):
                    attn_step(hp, qc, ki, avpA, avpB,
                              n == 0, n == len(ki_order) - 1)
                    if n == 0 and pending is not None:
                        rcps = norm_rcp(pending[1])
                    if n == 2 and pending is not None:
                        ph = pending[0]
                        norm_post(ph, 0, qc, rcps, pending[1])
                        norm_post(ph, 64, qc, rcps, pending[1])
                        pending = None
                    acc += pump_rate
                    while acc >= 1.0:
                        acc -= 1.0
                        if gen is not None:
                            if next(gen, StopIteration) is StopIteration:
                                gen = None
                            else:
                                continue
                        if filler:
                            filler.popleft()()
                pending = (hp, norm_copy(avpA, avpB))
            if pending is not None:
                rcps = norm_rcp(pending[1])
                # keep the PE fed while the rcp chain runs on DVE
                for _ in range(4):
                    if filler:
                        filler.popleft()()
                ph = pending[0]
                norm_post(ph, 0, qc, rcps, pending[1])
                norm_post(ph, 64, qc, rcps, pending[1])
            if gen is not None:
                for _ in gen:
                    pass
            for st in range(4 * qc, 4 * qc + 4):
                enqueue_oproj(qc, st)
        while filler:
            filler.popleft()()

    nc.compile()
    return nc


def _host_tables():
    inv_freq = 1.0 / (10000.0 ** (np.arange(0, HD, 2, dtype=np.float32) / HD))
    pos = np.arange(S, dtype=np.float32)
    freqs = np.einsum('s,d->sd', pos, inv_freq)          # [S, 32]
    emb = np.concatenate([freqs, freqs], axis=-1)        # [S, 64]
    cosT = np.cos(emb).T.astype(np.float32)              # [64, S]
    sinT = np.sin(emb).T.astype(np.float32)
    c2 = np.ascontiguousarray(np.vstack([cosT, cosT]))   # [128, S]
    ss = np.ascontiguousarray(np.vstack([sinT, sinT]))
    # rotate-half as a matmul: out[d] = sum_d' R[d', d] * in[d']
    R64 = np.zeros((HD, HD), dtype=np.float32)
    for d in range(32):
        R64[d + 32, d] = -1.0       # out[d] = -in[d+32]
        R64[d, d + 32] = 1.0        # out[d+32] = in[d]
    rot = np.zeros((P, P), dtype=np.float32)
    rot[0:64, 0:64] = R64
    rot[64:128, 64:128] = R64
    # causal bias for a diagonal 128x128 tile in scores^T[k, q] layout
    kk = np.arange(P)[:, None]
    qq = np.arange(P)[None, :]
    msk = np.where(kk <= qq, 0.0, MASK_VAL).astype(np.float32)
    import ml_dtypes
    rot = rot.astype(ml_dtypes.bfloat16)   # exact: entries are 0/+-1
    return c2, ss, rot, msk


# q/o head order within a rank block: pair heads (u, u+4) in each 128-row tile
_HEAD_ORDER = [0, 4, 1, 5, 2, 6, 3, 7]


def _pack128(a):
    """[128*n, m] row-major -> [128, n*m] with block i at cols [m*i, m*(i+1))."""
    n = a.shape[0] // P
    return np.ascontiguousarray(
        a.reshape(n, P, a.shape[1]).transpose(1, 0, 2).reshape(P, -1))


def _make_in_maps(hidden_states, Wq, Wk, Wv, Wo):
    import ml_dtypes
    BF = ml_dtypes.bfloat16
    hs = np.asarray(hidden_states, dtype=np.float32)
    Wq = np.asarray(Wq, dtype=np.float32)
    Wk = np.asarray(Wk, dtype=np.float32)
    Wv = np.asarray(Wv, dtype=np.float32)
    Wo = np.asarray(Wo, dtype=np.float32)
    c2, ss, rot, msk = _host_tables()
    in_maps = []
    for c in range(8):
        b, r = c // 4, c % 4
        # row indices of Wq (= cols of Wo) for this rank, in device head order
        qrows = np.concatenate([
            np.arange(HD) + (NHL * r + u) * HD for u in _HEAD_ORDER
        ])
        xt_ = hs[b].T.astype(BF)                          # [H, S]
        # chunk-major packing: [128, sc, i, s']
        xp = xt_.reshape(NHT, P, NSC, QC).transpose(1, 2, 0, 3).reshape(P, -1)
        wq_ = _pack128(Wq[qrows, :].T.astype(BF))         # [128, 16*512]
        wk_ = Wk[NKO * r:NKO * (r + 1), :].T.astype(BF)   # [H, 128]
        wv_ = Wv[NKO * r:NKO * (r + 1), :].T.astype(BF)
        wkv_ = _pack128(np.concatenate([wk_, wv_], axis=1))
        wot_ = _pack128(Wo[:, qrows].T.astype(BF))        # [128, 4*2048]
        in_maps.append({
            "xall": np.ascontiguousarray(xp),
            "wqall": wq_,
            "wkv": wkv_,
            "wot": wot_,
            "c2": c2, "ss": ss, "msk": msk, "rot": rot,
        })
    return in_maps


_NC = None


def _get_nc():
    global _NC
    if _NC is None:
        _NC = build_nc()
    return _NC


def run_cores(hidden_states, Wq, Wk, Wv, Wo, **run_kwargs):
    """Run the SPMD kernel; returns (out [B,S,H] fp32, BassKernelResults)."""
    nc = _get_nc()
    in_maps = _make_in_maps(hidden_states, Wq, Wk, Wv, Wo)
    res = run_bass_kernel_spmd(nc, in_maps, list(range(8)), **run_kwargs)
    out = np.zeros((B, S, H), dtype=np.float32)
    for c in range(8):
        yb = np.asarray(res.results[c]["y"], dtype=np.float32)
        out[c // 4] += yb.reshape(P, NPT, S).transpose(1, 0, 2).reshape(S, H)
    return out, res


def kernel(hidden_states, Wq, Wk, Wv, Wo):
    out, _ = run_cores(hidden_states, Wq, Wk, Wv, Wo)
    return out
